# revision 1
# baseline (speedup 1.0000x reference)
"""GAT layer (PyG GATConv eval, 2 heads x 128, self-loops, ELU) on 8 trn2 cores.

Strategy (dst-sharded, per core):
  Phase A: per 128-node tile of full h: PE-transpose -> matmul with Wa4
           (=W.att contractions) -> write a_src to DRAM table TA[N,64] (256B rows).
  Phase A-bis: same on this core's dst shard -> a_dst resident in SBUF.
  Phase B: edges grouped by dst block (128 dsts), chunks of 128 edges.
           dma_gather of h rows (512B) + TA rows (256B) by src id (int16
           lo/hi table split).  Per chunk: dst one-hot masks via iota+is_equal,
           ex = exp(leakyrelu(a_src+a_dst)), GT_h[k,d] += (ex_h*Hg)^T M (PE),
           segsum[d,h] += M^T ex (PE).  Per block: U_h = (GT_h)^T W_h via PE,
           normalize by segsum, +bias, ELU, write out rows.
Softmax max-subtraction is skipped (shift-invariant; logits are O(10) so fp32
exp cannot overflow).
"""
import math
from contextlib import ExitStack

import numpy as np

HEADS = 2
C = 128
IN = 128
N = 50000
NC_CORES = 8
SH = N // NC_CORES            # 6250 dst nodes per core
NBLK = math.ceil(SH / 128)    # 49 dst blocks per core
SHP = NBLK * 128              # padded shard rows 6272
NTILE = math.ceil(N / 128)    # 391 tiles of full h
LO = 32768                    # int16 gather index split
GRP = 4                       # dst blocks per dma_gather call group
NEG_SLOPE = 0.2

_CACHE = {}


# ----------------------------------------------------------------- host prep
def _prep(edge_index):
    src = np.concatenate([edge_index[0], np.arange(N, dtype=np.int64)])
    dst = np.concatenate([edge_index[1], np.arange(N, dtype=np.int64)])
    src = src.astype(np.int64)
    core = dst // SH
    blk = (dst % SH) // 128
    dloc = (dst % SH) % 128
    half = (src >= LO).astype(np.int64)          # 0 = lo, 1 = hi

    key = (core * NBLK + blk) * 2 + half
    order = np.argsort(key, kind="stable")
    key_s = key[order]
    src_s = src[order]
    dloc_s = dloc[order]

    ngroups = NC_CORES * NBLK * 2
    sizes = np.bincount(key_s, minlength=ngroups)
    starts = np.concatenate([[0], np.cumsum(sizes)[:-1]])
    rank = np.arange(len(key_s)) - starts[key_s]

    lo_sizes = sizes.reshape(NC_CORES, NBLK, 2)[:, :, 0]
    hi_sizes = sizes.reshape(NC_CORES, NBLK, 2)[:, :, 1]
    K_LO = int(math.ceil(lo_sizes.max() / 128))
    K_HI = int(math.ceil(hi_sizes.max() / 128))
    K = K_LO + K_HI

    half_s = key_s % 2
    slot = rank + half_s * (K_LO * 128)          # slot within block [0, K*128)

    core_s = key_s // (2 * NBLK)
    blk_s = (key_s // 2) % NBLK

    # int16 gather index per slot (dummy 0 for padding), dst_local (999 pad)
    idx16 = np.zeros((NC_CORES, NBLK, K * 128), dtype=np.int16)
    dstl = np.full((NC_CORES, NBLK, K * 128), 999.0, dtype=np.float32)
    idxv = np.where(half_s == 0, src_s, src_s - LO).astype(np.int16)
    idx16[core_s, blk_s, slot] = idxv
    dstl[core_s, blk_s, slot] = dloc_s.astype(np.float32)

    # wrapped-16 gather index layout per block: w[b, p, col] = idx[col*16 + p%16]
    sl = idx16.reshape(NC_CORES, NBLK, K * 8, 16)       # [., ., col, p16]
    w_lo = np.ascontiguousarray(
        np.broadcast_to(
            sl[:, :, : K_LO * 8].transpose(0, 1, 3, 2)[:, :, None, :, :],
            (NC_CORES, NBLK, 8, 16, K_LO * 8),
        ).reshape(NC_CORES, NBLK, 128, K_LO * 8)
    )
    w_hi = np.ascontiguousarray(
        np.broadcast_to(
            sl[:, :, K_LO * 8 :].transpose(0, 1, 3, 2)[:, :, None, :, :],
            (NC_CORES, NBLK, 8, 16, K_HI * 8),
        ).reshape(NC_CORES, NBLK, 128, K_HI * 8)
    )

    # dstl layouts: [b, p, j] (slot s = j*128+p); uint16 variant for the
    # per-block a_dst table lookup (pads point at entry 127)
    d3 = dstl.reshape(NC_CORES, NBLK, K, 128)           # [., ., j, p]
    dstl_pj = np.ascontiguousarray(d3.transpose(0, 1, 3, 2))  # [., ., p, j]
    dstl_jp = np.ascontiguousarray(d3)                  # [., ., j, p]
    return K_LO, K_HI, w_lo, w_hi, dstl_pj, dstl_jp


# ------------------------------------------------------------ device program
def _build(K_LO, K_HI, phase="full"):
    import concourse.bacc as bacc
    import concourse.bass as bass
    import concourse.mybir as mybir
    import concourse.tile as tile
    from concourse.masks import make_identity

    dt = mybir.dt
    op = mybir.AluOpType
    act = mybir.ActivationFunctionType
    K = K_LO + K_HI
    P = 128

    nc = bacc.Bacc("TRN2", target_bir_lowering=False, debug=False,
                   num_devices=NC_CORES)
    h = nc.dram_tensor("h", [N, IN], dt.float32, kind="ExternalInput")
    h_sh = nc.dram_tensor("h_sh", [SHP, IN], dt.float32, kind="ExternalInput")
    w_in = nc.dram_tensor("w_in", [IN, HEADS * C], dt.float32, kind="ExternalInput")
    asrc_in = nc.dram_tensor("asrc_in", [HEADS, C], dt.float32, kind="ExternalInput")
    adst_in = nc.dram_tensor("adst_in", [HEADS, C], dt.float32, kind="ExternalInput")
    bias_in = nc.dram_tensor("bias_in", [1, HEADS * C], dt.float32, kind="ExternalInput")
    wlo_in = nc.dram_tensor("wlo", [NBLK, P, K_LO * 8], dt.int16, kind="ExternalInput")
    whi_in = nc.dram_tensor("whi", [NBLK, P, K_HI * 8], dt.int16, kind="ExternalInput")
    dpj_in = nc.dram_tensor("dpj", [NBLK * P, K], dt.float32, kind="ExternalInput")
    djp_in = nc.dram_tensor("djp", [NBLK, K * P], dt.float32, kind="ExternalInput")
    ta = nc.dram_tensor("ta", [NTILE * P, 64], dt.float32)
    out_t = nc.dram_tensor("out", [SHP, HEADS * C], dt.float32, kind="ExternalOutput")

    hap = h.ap()
    STAGE = 17  # phase-A tiles staged per TA write burst (391 = 23*17)

    with tile.TileContext(nc) as tc, ExitStack() as ctx:
        const = ctx.enter_context(tc.tile_pool(name="const", bufs=1))
        ctxA = ctx.enter_context(ExitStack())
        sbA = ctxA.enter_context(tc.tile_pool(name="sbA", bufs=3))
        stg = ctxA.enter_context(tc.tile_pool(name="stg", bufs=2))

        # ---- constants
        ident = const.tile([P, P], dt.float32)
        make_identity(nc, ident[:])
        iota_row = const.tile([P, P], dt.float32)
        nc.gpsimd.iota(iota_row[:], pattern=[[1, P]], base=0, channel_multiplier=0,
                       allow_small_or_imprecise_dtypes=True)
        iota_col4 = const.tile([P, 512], dt.float32)
        nc.gpsimd.iota(iota_col4[:], pattern=[[0, 512]], base=0, channel_multiplier=1,
                       allow_small_or_imprecise_dtypes=True)
        w_sb = const.tile([P, HEADS * C], dt.float32)
        nc.sync.dma_start(w_sb[:], w_in.ap()[:, :])

        ones_col = const.tile([P, 1], dt.float32)
        nc.gpsimd.memset(ones_col[:], 1.0)
        ones1 = const.tile([1, P], dt.float32)
        nc.gpsimd.memset(ones1[:], 1.0)
        bias_bc = const.tile([P, HEADS * C], dt.float32)
        nc.sync.dma_start(bias_bc[:], bass.AP(bias_in, 0, [[0, P], [1, HEADS * C]]))

        # Wa4[k, 0:2] = sum_c W[k, h*C+c]*att_src[h, c];  cols 2:4 for att_dst
        wa4 = const.tile([P, 4], dt.float32)
        tmp_pool = ctxA.enter_context(tc.tile_pool(name="watmp", bufs=2))
        for hd in range(HEADS):
            for j, attt in enumerate((asrc_in, adst_in)):
                abc = tmp_pool.tile([P, C], dt.float32, tag="abc")
                nc.sync.dma_start(abc[:], bass.AP(attt, hd * C, [[0, P], [1, C]]))
                t = tmp_pool.tile([P, C], dt.float32, tag="t")
                nc.vector.tensor_tensor(
                    out=t[:], in0=w_sb[:, hd * C:(hd + 1) * C],
                    in1=abc[:], op=op.mult)
                nc.vector.tensor_reduce(
                    out=wa4[:, 2 * j + hd:2 * j + hd + 1], in_=t[:],
                    axis=mybir.AxisListType.X, op=op.add)

        psA = ctxA.enter_context(tc.tile_pool(name="psA", bufs=2, space="PSUM"))
        psA2 = ctxA.enter_context(tc.tile_pool(name="psA2", bufs=2, space="PSUM"))

        # ---- phase A: a_src table for all N (+ phase A-bis shard a_dst)
        adst_sb = const.tile([P, NBLK, 2], dt.float32)

        def attn_tile(src_ap, nrows):
            """load [nrows,128] h rows -> return psum [128,4] a-values tile."""
            ht = sbA.tile([P, IN], dt.float32, tag="ht")
            nc.sync.dma_start(ht[:nrows, :], src_ap)
            tp = psA.tile([P, P], dt.float32, tag="tp", space="PSUM")
            nc.tensor.transpose(out=tp[:], in_=ht[:], identity=ident[:])
            hT = sbA.tile([P, P], dt.float32, tag="hT")
            nc.scalar.copy(out=hT[:], in_=tp[:])
            a4 = psA2.tile([P, 4], dt.float32, tag="a4", space="PSUM")
            nc.tensor.matmul(out=a4[:], lhsT=hT[:], rhs=wa4[:], start=True, stop=True)
            return a4

        for t0 in range(0, NTILE, STAGE):
            nst = min(STAGE, NTILE - t0)
            st = stg.tile([P, STAGE, 4], dt.float32, tag="st")
            for g in range(nst):
                ti = t0 + g
                nrows = min(P, N - ti * P)
                a4 = attn_tile(hap[ti * P:ti * P + nrows, :], nrows)
                nc.scalar.copy(out=st[:, g, :], in_=a4[:])
            # burst write to TA rows [t0*128, (t0+nst)*128), cols 0:4
            out_ap = bass.AP(ta, t0 * P * 64, [[64, P], [P * 64, nst], [1, 4]])
            nc.gpsimd.dma_start(out_ap, st[:, :nst, :])

        for b in range(NBLK):
            a4 = attn_tile(h_sh.ap()[b * P:(b + 1) * P, :], P)
            nc.scalar.copy(out=adst_sb[:, b, 0:2], in_=a4[:, 2:4])

        ctxA.close()  # free phase-A SBUF + PSUM pools before phase B

        # ---- phase B
        gh = ctx.enter_context(tc.tile_pool(name="gh", bufs=2))
        gt = ctx.enter_context(tc.tile_pool(name="gt", bufs=2))
        gi = ctx.enter_context(tc.tile_pool(name="gi", bufs=2))
        mk = ctx.enter_context(tc.tile_pool(name="mk", bufs=3))
        sm = ctx.enter_context(tc.tile_pool(name="sm", bufs=3))
        fin = ctx.enter_context(tc.tile_pool(name="fin", bufs=2))
        psGT = ctx.enter_context(tc.tile_pool(name="psGT", bufs=2, space="PSUM"))
        psSS = ctx.enter_context(tc.tile_pool(name="psSS", bufs=1, space="PSUM"))
        psAD = ctx.enter_context(tc.tile_pool(name="psAD", bufs=1, space="PSUM"))
        psB = ctx.enter_context(tc.tile_pool(name="psB", bufs=1, space="PSUM"))
        psU = ctx.enter_context(tc.tile_pool(name="psU", bufs=1, space="PSUM"))

        taap = ta.ap()
        blk_range = [] if phase == "A" else list(range(0, NBLK, GRP))
        for g0 in blk_range:
            ng = min(GRP, NBLK - g0)
            ilo = gi.tile([P, GRP * K_LO * 8], dt.int16, tag="ilo")
            nc.sync.dma_start(
                ilo[:, : ng * K_LO * 8],
                bass.AP(wlo_in, g0 * P * K_LO * 8,
                        [[K_LO * 8, P], [P * K_LO * 8, ng], [1, K_LO * 8]]))
            ihi = gi.tile([P, GRP * K_HI * 8], dt.int16, tag="ihi")
            nc.sync.dma_start(
                ihi[:, : ng * K_HI * 8],
                bass.AP(whi_in, g0 * P * K_HI * 8,
                        [[K_HI * 8, P], [P * K_HI * 8, ng], [1, K_HI * 8]]))

            hg_lo = gh.tile([P, GRP * K_LO, IN], dt.float32, tag="hglo")
            nc.gpsimd.dma_gather(
                out_ap=hg_lo[:, : ng * K_LO, :], in_ap=hap[0:LO, :],
                idxs_ap=ilo[:, : ng * K_LO * 8], num_idxs=ng * K_LO * P,
                num_idxs_reg=ng * K_LO * P, elem_size=IN, single_packet=False)
            hg_hi = gh.tile([P, GRP * K_HI, IN], dt.float32, tag="hghi")
            nc.gpsimd.dma_gather(
                out_ap=hg_hi[:, : ng * K_HI, :], in_ap=hap[LO:N, :],
                idxs_ap=ihi[:, : ng * K_HI * 8], num_idxs=ng * K_HI * P,
                num_idxs_reg=ng * K_HI * P, elem_size=IN, single_packet=False)
            ta_lo = gt.tile([P, GRP * K_LO, 64], dt.float32, tag="talo")
            nc.gpsimd.dma_gather(
                out_ap=ta_lo[:, : ng * K_LO, :], in_ap=taap[0:LO, :],
                idxs_ap=ilo[:, : ng * K_LO * 8], num_idxs=ng * K_LO * P,
                num_idxs_reg=ng * K_LO * P, elem_size=64, single_packet=False)
            ta_hi = gt.tile([P, GRP * K_HI, 64], dt.float32, tag="tahi")
            nc.gpsimd.dma_gather(
                out_ap=ta_hi[:, : ng * K_HI, :], in_ap=taap[LO:NTILE * P, :],
                idxs_ap=ihi[:, : ng * K_HI * 8], num_idxs=ng * K_HI * P,
                num_idxs_reg=ng * K_HI * P, elem_size=64, single_packet=False)

            if phase == "gather":
                ob0 = fin.tile([P, HEADS * C], dt.float32, tag="ob")
                nc.vector.tensor_copy(out=ob0[:, 0:IN], in_=hg_lo[:, 0, :])
                nc.vector.tensor_copy(out=ob0[:, IN:IN + 64], in_=ta_lo[:, 0, :])
                nc.vector.tensor_copy(out=ob0[:, IN + 64:IN + 128],
                                      in_=ta_hi[:, 0, :])
                nc.sync.dma_start(out_t.ap()[(g0 // GRP) * P:(g0 // GRP + 1) * P, :], ob0[:])
                continue
            for bg in range(ng):
                b = g0 + bg
                dpj = sm.tile([P, K], dt.float32, tag="dpj")
                nc.sync.dma_start(dpj[:], dpj_in.ap()[b * P:(b + 1) * P, :])
                djp = sm.tile([1, K * P], dt.float32, tag="djp")
                nc.sync.dma_start(djp[:], djp_in.ap()[b:b + 1, :])

                # a_dst per edge slot via transposed one-hot matmuls,
                # 4 chunks per broadcast round
                adp = psAD.tile([P, K, 2], dt.float32, tag="adp", space="PSUM")
                for j0 in range(0, K, 4):
                    nb = min(4, K - j0)
                    bc = psB.tile([P, 512], dt.float32, tag="bc", space="PSUM")
                    nc.tensor.matmul(out=bc[:, : nb * P], lhsT=ones1[:],
                                     rhs=djp[:, j0 * P:(j0 + nb) * P],
                                     start=True, stop=True)
                    mt4 = mk.tile([P, 512], dt.float32, tag="mt4")
                    nc.vector.tensor_tensor(out=mt4[:, : nb * P],
                                            in0=iota_col4[:, : nb * P],
                                            in1=bc[:, : nb * P], op=op.is_equal)
                    for jj in range(nb):
                        nc.tensor.matmul(out=adp[:, j0 + jj, :],
                                         lhsT=mt4[:, jj * P:(jj + 1) * P],
                                         rhs=adst_sb[:, b, :],
                                         start=True, stop=True)

                # logits -> ex for every slot of the block, batched wide ops
                tsum = sm.tile([P, K, 2], dt.float32, tag="tsum")
                nc.vector.tensor_tensor(
                    out=tsum[:, :K_LO, :],
                    in0=ta_lo[:, bg * K_LO:(bg + 1) * K_LO, 0:2],
                    in1=adp[:, :K_LO, :], op=op.add)
                nc.vector.tensor_tensor(
                    out=tsum[:, K_LO:, :],
                    in0=ta_hi[:, bg * K_HI:(bg + 1) * K_HI, 0:2],
                    in1=adp[:, K_LO:, :], op=op.add)
                u02 = sm.tile([P, K, 2], dt.float32, tag="u02")
                nc.vector.tensor_scalar(out=u02[:], in0=tsum[:], scalar1=NEG_SLOPE,
                                        scalar2=None, op0=op.mult)
                lr = sm.tile([P, K, 2], dt.float32, tag="lr")
                nc.vector.tensor_tensor(out=lr[:], in0=tsum[:], in1=u02[:],
                                        op=op.max)
                ex = sm.tile([P, K, 2], dt.float32, tag="ex")
                nc.scalar.activation(out=ex[:], in_=lr[:], func=act.Exp)

                gtt = psGT.tile([P, 2 * P], dt.float32, tag="gt", space="PSUM")
                ss0 = psSS.tile([P, 1], dt.float32, tag="ss0", space="PSUM")
                ss1 = psSS.tile([P, 1], dt.float32, tag="ss1", space="PSUM")

                for j in range(K):
                    if j < K_LO:
                        hgc = hg_lo[:, bg * K_LO + j, :]
                    else:
                        hgc = hg_hi[:, bg * K_HI + (j - K_LO), :]
                    st_ = j == 0
                    sp = j == K - 1
                    exm = mk.tile([P, 2 * P], dt.float32, tag="exm")
                    for hd, sstile in ((0, ss0), (1, ss1)):
                        nc.vector.tensor_scalar(
                            out=exm[:, hd * P:(hd + 1) * P], in0=iota_row[:],
                            scalar1=dpj[:, j:j + 1],
                            scalar2=ex[:, j, hd:hd + 1],
                            op0=op.is_equal, op1=op.mult)
                        nc.tensor.matmul(out=sstile[:],
                                         lhsT=exm[:, hd * P:(hd + 1) * P],
                                         rhs=ones_col[:], start=st_, stop=sp)
                    nc.tensor.matmul(out=gtt[:], lhsT=hgc, rhs=exm[:],
                                     start=st_, stop=sp)

                # ---- finalize block b
                rec = fin.tile([P, 2], dt.float32, tag="rec")
                nc.vector.reciprocal(out=rec[:, 0:1], in_=ss0[:])
                nc.vector.reciprocal(out=rec[:, 1:2], in_=ss1[:])
                ob = fin.tile([P, HEADS * C], dt.float32, tag="ob")
                for hd in range(HEADS):
                    gs = fin.tile([P, P], dt.float32, tag="gs")
                    nc.scalar.copy(out=gs[:], in_=gtt[:, hd * P:(hd + 1) * P])
                    u = psU.tile([P, P], dt.float32, tag="u", space="PSUM")
                    nc.tensor.matmul(out=u[:], lhsT=gs[:],
                                     rhs=w_sb[:, hd * C:(hd + 1) * C],
                                     start=True, stop=True)
                    o = fin.tile([P, C], dt.float32, tag="o")
                    nc.vector.tensor_scalar(
                        out=o[:], in0=u[:], scalar1=rec[:, hd:hd + 1],
                        scalar2=None, op0=op.mult)
                    o2 = fin.tile([P, C], dt.float32, tag="o2")
                    nc.vector.tensor_tensor(
                        out=o2[:], in0=o[:],
                        in1=bias_bc[:, hd * C:(hd + 1) * C], op=op.add)
                    a1 = fin.tile([P, C], dt.float32, tag="a1")
                    nc.vector.tensor_scalar(out=a1[:], in0=o2[:], scalar1=0.0,
                                            scalar2=None, op0=op.min)
                    e1 = fin.tile([P, C], dt.float32, tag="e1")
                    nc.scalar.activation(out=e1[:], in_=a1[:], func=act.Exp)
                    a3 = fin.tile([P, C], dt.float32, tag="a3")
                    nc.vector.tensor_scalar(out=a3[:], in0=o2[:], scalar1=0.0,
                                            scalar2=-1.0, op0=op.max, op1=op.add)
                    nc.vector.tensor_tensor(
                        out=ob[:, hd * C:(hd + 1) * C], in0=a3[:], in1=e1[:],
                        op=op.add)
                nc.sync.dma_start(out_t.ap()[b * P:(b + 1) * P, :], ob[:])

    nc.compile()
    return nc


def _get_program(K_LO, K_HI):
    key = (K_LO, K_HI)
    if key not in _CACHE:
        _CACHE[key] = _build(K_LO, K_HI)
    return _CACHE[key]


# ------------------------------------------------------------------- kernel
def kernel(h_node, edge_index, W, att_src, att_dst, bias):
    from concourse.bass_utils import run_bass_kernel_spmd

    h_node = np.asarray(h_node, dtype=np.float32)
    W = np.asarray(W, dtype=np.float32)
    att_src = np.asarray(att_src, dtype=np.float32)
    att_dst = np.asarray(att_dst, dtype=np.float32)
    bias = np.asarray(bias, dtype=np.float32).reshape(1, HEADS * C)

    K_LO, K_HI, w_lo, w_hi, dstl_pj, dstl_jp = _prep(np.asarray(edge_index))
    nc = _get_program(K_LO, K_HI)

    in_maps = []
    for c in range(NC_CORES):
        hs = np.zeros((SHP, IN), dtype=np.float32)
        hs[:SH] = h_node[c * SH:(c + 1) * SH]
        in_maps.append({
            "h": h_node, "h_sh": hs, "w_in": W, "asrc_in": att_src,
            "adst_in": att_dst, "bias_in": bias,
            "wlo": w_lo[c], "whi": w_hi[c],
            "dpj": dstl_pj[c].reshape(NBLK * 128, K_LO + K_HI),
            "djp": dstl_jp[c].reshape(NBLK, (K_LO + K_HI) * 128),
        })
    res = run_bass_kernel_spmd(nc, in_maps, core_ids=list(range(NC_CORES)))
    out = np.concatenate([res.results[c]["out"][:SH] for c in range(NC_CORES)], axis=0)
    return out



# revision 10
# speedup vs baseline: 1.9569x; 1.9569x over previous
"""GAT layer (PyG GATConv eval, 2 heads x 128, self-loops, ELU) on 8 trn2 cores.

v2 strategy (dst-sharded, per core):
  hpack[N,128] f32 rows (512B): cols 0:64 = h as packed bf16 pairs (host),
  cols 64:66 = a_src logits f32 (device phase A embeds).  ONE dma_gather by
  src id per edge slot fetches h (bf16) + a_src together.
  Phase A: a4 = hT_tile^T @ wa4 (host passes hT bf16, no PE transposes),
  embed a_src into hpack; local-shard a_dst kept in SBUF (h_shT input).
  Phase B: edges grouped by dst block (128 dsts), chunks of 128 slots,
  per-block chunk counts = max over cores (not global max).  Per chunk:
  exm = fused is_equal+mult masks (bf16), gtt += hg^T @ exm (bf16 PE),
  segsum via exm^T @ ones, a_dst per slot via mask-transpose matmuls.
  Finalize: U = GT^T W (bf16), normalize, +bias, exact ELU.
"""
import math
from contextlib import ExitStack

import numpy as np
import ml_dtypes

BF16 = ml_dtypes.bfloat16
HEADS = 2
C = 128
IN = 128
N = 50000
NC_CORES = 8
SH = N // NC_CORES            # 6250 dst nodes per core
NBLK = math.ceil(SH / 128)    # 49 dst blocks per core
SHP = NBLK * 128              # padded shard rows 6272
NTILE = math.ceil(N / 128)    # 391 tiles of full h
NPAD = NTILE * 128            # 50048 padded rows of hpack
LO = 32768                    # int16 gather index split
RL = 40                       # max lo chunks per gather run
RH = 24                       # max hi chunks per gather run
NEG_SLOPE = 0.2

_CACHE = {}


# ----------------------------------------------------------------- host prep
def _wrap16(idx, nchunk):
    """idx [nchunk*128] int16 -> wrapped gather table [128, nchunk*8]."""
    sl = idx.reshape(nchunk * 8, 16)            # [col, p16]
    w = np.broadcast_to(sl.T[None, :, :], (8, 16, nchunk * 8))
    return np.ascontiguousarray(w.reshape(128, nchunk * 8))


def _prep(edge_index):
    src = np.concatenate([edge_index[0], np.arange(N, dtype=np.int64)])
    dst = np.concatenate([edge_index[1], np.arange(N, dtype=np.int64)])
    core = dst // SH
    blk = (dst % SH) // 128
    dloc = (dst % SH) % 128
    half = (src >= LO).astype(np.int64)

    # per (core, block, half) counts -> per-block chunk counts (max over cores)
    cnt = np.zeros((NC_CORES, NBLK, 2), dtype=np.int64)
    np.add.at(cnt, (core, blk, half), 1)
    KL = np.maximum(np.ceil(cnt[:, :, 0] / 128).astype(np.int64).max(0), 0)
    KH = np.maximum(np.ceil(cnt[:, :, 1] / 128).astype(np.int64).max(0), 0)
    offL = np.concatenate([[0], np.cumsum(KL)])   # lo-stream chunk offsets
    offH = np.concatenate([[0], np.cumsum(KH)])
    NL, NH = int(offL[-1]), int(offH[-1])

    # slot assignment: stable sort by (core, blk, half); rank within group
    key = (core * NBLK + blk) * 2 + half
    order = np.argsort(key, kind="stable")
    key_s = key[order]
    sizes = np.bincount(key_s, minlength=NC_CORES * NBLK * 2)
    starts = np.concatenate([[0], np.cumsum(sizes)[:-1]])
    rank = np.arange(len(key_s)) - starts[key_s]
    src_s = src[order]
    dloc_s = dloc[order]
    core_s = key_s // (2 * NBLK)
    blk_s = (key_s // 2) % NBLK
    half_s = key_s % 2

    # global slot position within each core's lo/hi stream
    strm_off = np.where(half_s == 0, offL[blk_s] * 128, offH[blk_s] * 128)
    slot = strm_off + rank

    idxL = np.zeros((NC_CORES, NL * 128), dtype=np.int16)
    idxH = np.zeros((NC_CORES, NH * 128), dtype=np.int16)
    dpjL = np.full((NC_CORES, NL, 128), 999.0, dtype=np.float32)
    dpjH = np.full((NC_CORES, NH, 128), 999.0, dtype=np.float32)

    lo_m = half_s == 0
    idxL[core_s[lo_m], slot[lo_m]] = src_s[lo_m].astype(np.int16)
    idxH[core_s[~lo_m], slot[~lo_m]] = (src_s[~lo_m] - LO).astype(np.int16)
    dpjL[core_s[lo_m], slot[lo_m] // 128, slot[lo_m] % 128] = dloc_s[lo_m]
    dpjH[core_s[~lo_m], slot[~lo_m] // 128, slot[~lo_m] % 128] = dloc_s[~lo_m]

    wlo = np.stack([_wrap16(idxL[c], NL) for c in range(NC_CORES)])
    whi = np.stack([_wrap16(idxH[c], NH) for c in range(NC_CORES)])
    # dpj tables [128 partitions, nchunk] (scalar per partition per chunk)
    dpjL_t = np.ascontiguousarray(dpjL.transpose(0, 2, 1))
    dpjH_t = np.ascontiguousarray(dpjH.transpose(0, 2, 1))
    # djp rows [1, nchunk*128] bf16 for the PE broadcast matmul
    djpL = dpjL.reshape(NC_CORES, NL * 128).astype(BF16)
    djpH = dpjH.reshape(NC_CORES, NH * 128).astype(BF16)

    # gather runs: greedy whole blocks with sum KL<=RL and sum KH<=RH
    runs = []
    b = 0
    while b < NBLK:
        b1 = b + 1
        while b1 < NBLK and (KL[b:b1 + 1].sum() <= RL and KH[b:b1 + 1].sum() <= RH):
            b1 += 1
        runs.append((b, b1))
        b = b1
    params = (tuple(int(k) for k in KL), tuple(int(k) for k in KH),
              tuple(runs))
    return params, wlo, whi, dpjL_t, dpjH_t, djpL, djpH


def _pack_inputs(h_node, W, att_src, att_dst, bias):
    hb = h_node.astype(BF16)                       # [N,128] bf16
    hpack = np.zeros((NPAD, 128), dtype=np.float32)
    hpack[:N, 0:64] = hb.view(np.uint16).reshape(N, 64, 2).view(np.uint32).reshape(N, 64).view(np.float32)
    hT = np.zeros((128, NPAD), dtype=BF16)
    hT[:, :N] = hb.T
    h_shT = np.zeros((NC_CORES, 128, SHP), dtype=BF16)
    for c in range(NC_CORES):
        h_shT[c, :, :SH] = hb[c * SH:(c + 1) * SH].T
    W3 = W.reshape(IN, HEADS, C)
    wa4 = np.stack([
        np.einsum('cho,ho->c', W3, att_src * (np.arange(HEADS)[:, None] == 0)),
        np.einsum('cho,ho->c', W3, att_src * (np.arange(HEADS)[:, None] == 1)),
        np.einsum('cho,ho->c', W3, att_dst * (np.arange(HEADS)[:, None] == 0)),
        np.einsum('cho,ho->c', W3, att_dst * (np.arange(HEADS)[:, None] == 1)),
    ], axis=1).astype(BF16)                        # [128, 4]
    wsb = W.astype(BF16)                           # [128, 256]
    bias2 = bias.reshape(1, HEADS * C).astype(np.float32)
    return hpack, hT, h_shT, wa4, wsb, bias2


# ------------------------------------------------------------ device program
def _build(params):
    import concourse.bacc as bacc
    import concourse.bass as bass
    import concourse.mybir as mybir
    import concourse.tile as tile

    KL, KH, runs = params
    offL = [0]
    for k in KL:
        offL.append(offL[-1] + k)
    offH = [0]
    for k in KH:
        offH.append(offH[-1] + k)
    NL, NH = offL[-1], offH[-1]
    KMAX = max(KL[b] + KH[b] for b in range(NBLK))

    dt = mybir.dt
    op = mybir.AluOpType
    act = mybir.ActivationFunctionType
    P = 128

    nc = bacc.Bacc("TRN2", target_bir_lowering=False, debug=False,
                   num_devices=NC_CORES)
    hpack = nc.dram_tensor("hpack", [NPAD, 128], dt.float32, kind="ExternalInput")
    hT_in = nc.dram_tensor("hT", [128, NPAD], dt.bfloat16, kind="ExternalInput")
    hshT_in = nc.dram_tensor("hshT", [128, SHP], dt.bfloat16, kind="ExternalInput")
    wa4_in = nc.dram_tensor("wa4", [128, 4], dt.bfloat16, kind="ExternalInput")
    wsb_in = nc.dram_tensor("wsb", [128, HEADS * C], dt.bfloat16, kind="ExternalInput")
    bias_in = nc.dram_tensor("bias_in", [1, HEADS * C], dt.float32, kind="ExternalInput")
    wlo_in = nc.dram_tensor("wlo", [P, max(NL, 1) * 8], dt.int16, kind="ExternalInput")
    whi_in = nc.dram_tensor("whi", [P, max(NH, 1) * 8], dt.int16, kind="ExternalInput")
    dpjL_in = nc.dram_tensor("dpjL", [P, max(NL, 1)], dt.float32, kind="ExternalInput")
    dpjH_in = nc.dram_tensor("dpjH", [P, max(NH, 1)], dt.float32, kind="ExternalInput")
    djpL_in = nc.dram_tensor("djpL", [1, max(NL, 1) * 128], dt.bfloat16, kind="ExternalInput")
    djpH_in = nc.dram_tensor("djpH", [1, max(NH, 1) * 128], dt.bfloat16, kind="ExternalInput")
    out_t = nc.dram_tensor("out", [SHP, HEADS * C], dt.float32, kind="ExternalOutput")

    with tile.TileContext(nc) as tc, ExitStack() as ctx:
        const = ctx.enter_context(tc.tile_pool(name="const", bufs=1))

        # ---- constants
        iota_row_f = const.tile([P, P], dt.float32)
        nc.gpsimd.iota(iota_row_f[:], pattern=[[1, P]], base=0,
                       channel_multiplier=0, allow_small_or_imprecise_dtypes=True)
        iota_row = const.tile([P, P], dt.bfloat16)
        nc.vector.tensor_copy(out=iota_row[:], in_=iota_row_f[:])
        iota_col4 = const.tile([P, 512], dt.float32)
        nc.gpsimd.iota(iota_col4[:], pattern=[[0, 512]], base=0,
                       channel_multiplier=1, allow_small_or_imprecise_dtypes=True)
        ones1 = const.tile([1, P], dt.bfloat16)
        nc.gpsimd.memset(ones1[:], 1.0)
        ones_col = const.tile([P, 1], dt.bfloat16)
        nc.gpsimd.memset(ones_col[:], 1.0)
        wa4_sb = const.tile([P, 4], dt.bfloat16)
        nc.sync.dma_start(wa4_sb[:], wa4_in.ap()[:, :])
        wsb = const.tile([P, HEADS * C], dt.bfloat16)
        nc.sync.dma_start(wsb[:], wsb_in.ap()[:, :])
        bias_bc = const.tile([P, HEADS * C], dt.float32)
        nc.sync.dma_start(bias_bc[:], bass.AP(bias_in, 0, [[0, P], [1, HEADS * C]]))
        adst_sb = const.tile([P, NBLK, 2], dt.float32)
        adst_bf = const.tile([P, NBLK, 2], dt.bfloat16)

        # ---- phase B input tables (preload during phase A)
        wloT = const.tile([P, max(NL, 1) * 8], dt.int16)
        nc.sync.dma_start(wloT[:], wlo_in.ap()[:, :])
        whiT = const.tile([P, max(NH, 1) * 8], dt.int16)
        nc.sync.dma_start(whiT[:], whi_in.ap()[:, :])
        dpjL_sb = const.tile([P, max(NL, 1)], dt.float32)
        nc.sync.dma_start(dpjL_sb[:], dpjL_in.ap()[:, :])
        dpjH_sb = const.tile([P, max(NH, 1)], dt.float32)
        nc.sync.dma_start(dpjH_sb[:], dpjH_in.ap()[:, :])

        # ---- phase A: a4 = hT_tile^T @ wa4 for all N; embed a_src into hpack
        ctxA = ExitStack()
        sbA = ctxA.enter_context(tc.tile_pool(name="sbA", bufs=3))
        psA = ctxA.enter_context(tc.tile_pool(name="psA", bufs=3, space="PSUM"))
        st = const.tile([P, NTILE, 4], dt.float32)   # a4 staging, all tiles
        NT4 = NTILE // 4            # 391 = 97*4 + 3
        for t4 in range(NT4 + 1):
            nt = 4 if t4 < NT4 else NTILE - NT4 * 4
            if nt == 0:
                break
            ht = sbA.tile([P, 512], dt.bfloat16, tag="ht")
            nc.sync.dma_start(ht[:, :nt * 128],
                              hT_in.ap()[:, t4 * 512:t4 * 512 + nt * 128])
            a4p = psA.tile([P, 4, 4], dt.float32, tag="a4", space="PSUM")
            for g in range(nt):
                nc.tensor.matmul(out=a4p[:, g, :],
                                 lhsT=ht[:, g * 128:(g + 1) * 128],
                                 rhs=wa4_sb[:], start=True, stop=True)
            nc.vector.tensor_copy(out=st[:, t4 * 4:t4 * 4 + nt, :],
                                  in_=a4p[:, :nt, :])
        # embed a_src (cols 0:2 of a4) into hpack cols 64:66
        emb_ap = bass.AP(hpack, 64, [[128, P], [128 * 128, NTILE], [1, 2]])
        nc.sync.dma_start(emb_ap, st[:, :, 0:2])

        # phase A-bis: local shard a_dst from h_shT
        NB4 = NBLK // 4             # 49 = 12*4 + 1
        for t4 in range(NB4 + 1):
            nt = 4 if t4 < NB4 else NBLK - NB4 * 4
            if nt == 0:
                break
            ht = sbA.tile([P, 512], dt.bfloat16, tag="ht")
            nc.sync.dma_start(ht[:, :nt * 128],
                              hshT_in.ap()[:, t4 * 512:t4 * 512 + nt * 128])
            a4p = psA.tile([P, 4, 2], dt.float32, tag="a4b", space="PSUM")
            for g in range(nt):
                nc.tensor.matmul(out=a4p[:, g, :],
                                 lhsT=ht[:, g * 128:(g + 1) * 128],
                                 rhs=wa4_sb[:, 2:4], start=True, stop=True)
            nc.vector.tensor_copy(out=adst_sb[:, t4 * 4:t4 * 4 + nt, :],
                                  in_=a4p[:, :nt, :])
        nc.vector.tensor_copy(out=adst_bf[:], in_=adst_sb[:])
        ctxA.close()

        # ---- phase B
        ghL = ctx.enter_context(tc.tile_pool(name="ghL", bufs=2))
        ghH = ctx.enter_context(tc.tile_pool(name="ghH", bufs=2))
        gdj = ctx.enter_context(tc.tile_pool(name="gdj", bufs=2))
        mk = ctx.enter_context(tc.tile_pool(name="mk", bufs=3))
        sm = ctx.enter_context(tc.tile_pool(name="sm", bufs=2))
        fin = ctx.enter_context(tc.tile_pool(name="fin", bufs=2))
        psGT = ctx.enter_context(tc.tile_pool(name="psGT", bufs=2, space="PSUM"))
        psB = ctx.enter_context(tc.tile_pool(name="psB", bufs=1, space="PSUM"))
        psAD = ctx.enter_context(tc.tile_pool(name="psAD", bufs=2, space="PSUM"))
        psSS = ctx.enter_context(tc.tile_pool(name="psSS", bufs=1, space="PSUM"))
        psU = ctx.enter_context(tc.tile_pool(name="psU", bufs=1, space="PSUM"))

        hp_ap = hpack.ap()
        for (b0, b1) in runs:
            nL = offL[b1] - offL[b0]
            nH = offH[b1] - offH[b0]
            hgl = ghL.tile([P, RL, 128], dt.float32, tag="hgl")
            if nL:
                nc.gpsimd.dma_gather(
                    out_ap=hgl[:, :nL, :], in_ap=hp_ap[0:LO, :],
                    idxs_ap=wloT[:, offL[b0] * 8:offL[b1] * 8],
                    num_idxs=nL * P, num_idxs_reg=nL * P,
                    elem_size=128, single_packet=False)
            hgh = ghH.tile([P, RH, 128], dt.float32, tag="hgh")
            if nH:
                nc.gpsimd.dma_gather(
                    out_ap=hgh[:, :nH, :], in_ap=hp_ap[LO:NPAD, :],
                    idxs_ap=whiT[:, offH[b0] * 8:offH[b1] * 8],
                    num_idxs=nH * P, num_idxs_reg=nH * P,
                    elem_size=128, single_packet=False)
            djl = gdj.tile([1, RL * 128], dt.bfloat16, tag="djl")
            if nL:
                nc.sync.dma_start(djl[:, :nL * 128],
                                  djpL_in.ap()[:, offL[b0] * 128:offL[b1] * 128])
            djh = gdj.tile([1, RH * 128], dt.bfloat16, tag="djh")
            if nH:
                nc.sync.dma_start(djh[:, :nH * 128],
                                  djpH_in.ap()[:, offH[b0] * 128:offH[b1] * 128])

            for b in range(b0, b1):
                kl, kh = KL[b], KH[b]
                K = kl + kh
                if K == 0:
                    continue
                # chunk descriptors: (hg tile, col in tile, dpj table, pos)
                chunks = []
                for j in range(kl):
                    chunks.append((hgl, offL[b] - offL[b0] + j, dpjL_sb,
                                   offL[b] + j, djl, (offL[b] - offL[b0] + j)))
                for j in range(kh):
                    chunks.append((hgh, offH[b] - offH[b0] + j, dpjH_sb,
                                   offH[b] + j, djh, (offH[b] - offH[b0] + j)))

                # pass 1: a_dst per slot via mask-transpose matmuls
                adp = psAD.tile([P, KMAX, 2], dt.float32, tag="adp", space="PSUM")
                for g0 in range(0, K, 4):
                    ng = min(4, K - g0)
                    bcp = psB.tile([P, 512], dt.float32, tag="bc", space="PSUM")
                    for gg in range(ng):
                        _, _, _, _, djt, dcol = chunks[g0 + gg]
                        nc.tensor.matmul(
                            out=bcp[:, gg * 128:(gg + 1) * 128], lhsT=ones1[:],
                            rhs=djt[:, dcol * 128:(dcol + 1) * 128],
                            start=True, stop=True)
                    mt4 = mk.tile([P, 512], dt.bfloat16, tag="mt4")
                    nc.vector.tensor_tensor(
                        out=mt4[:, :ng * 128], in0=iota_col4[:, :ng * 128],
                        in1=bcp[:, :ng * 128], op=op.is_equal)
                    for gg in range(ng):
                        nc.tensor.matmul(
                            out=adp[:, g0 + gg, :],
                            lhsT=mt4[:, gg * 128:(gg + 1) * 128],
                            rhs=adst_bf[:, b, :], start=True, stop=True)

                # logits -> ex  [P, K, 2]
                tsum = sm.tile([P, KMAX, 2], dt.float32, tag="tsum")
                if kl:
                    nc.vector.tensor_tensor(
                        out=tsum[:, :kl, :],
                        in0=hgl[:, offL[b] - offL[b0]:offL[b] - offL[b0] + kl, 64:66],
                        in1=adp[:, :kl, :], op=op.add)
                if kh:
                    nc.vector.tensor_tensor(
                        out=tsum[:, kl:K, :],
                        in0=hgh[:, offH[b] - offH[b0]:offH[b] - offH[b0] + kh, 64:66],
                        in1=adp[:, kl:K, :], op=op.add)
                u02 = sm.tile([P, KMAX, 2], dt.float32, tag="u02")
                nc.vector.tensor_scalar(out=u02[:, :K, :], in0=tsum[:, :K, :],
                                        scalar1=NEG_SLOPE, scalar2=None, op0=op.mult)
                lr = sm.tile([P, KMAX, 2], dt.float32, tag="lr")
                nc.vector.tensor_tensor(out=lr[:, :K, :], in0=tsum[:, :K, :],
                                        in1=u02[:, :K, :], op=op.max)
                ex2 = sm.tile([P, KMAX, 2], dt.float32, tag="ex2")
                nc.scalar.activation(out=ex2[:, :K, :], in_=lr[:, :K, :], func=act.Exp)

                # pass 2: masked scatter matmuls
                gtt = psGT.tile([P, HEADS * P], dt.float32, tag="gtt", space="PSUM")
                ss0 = psSS.tile([P, 1], dt.float32, tag="ss0", space="PSUM")
                ss1 = psSS.tile([P, 1], dt.float32, tag="ss1", space="PSUM")
                for k, (hg, col, dpjt, dpos, _, _) in enumerate(chunks):
                    st_, sp_ = k == 0, k == K - 1
                    exm = mk.tile([P, 2 * P], dt.bfloat16, tag="exm")
                    nc.vector.tensor_scalar(
                        out=exm[:, 0:P], in0=iota_row[:],
                        scalar1=dpjt[:, dpos:dpos + 1],
                        scalar2=ex2[:, k, 0:1], op0=op.is_equal, op1=op.mult)
                    nc.vector.tensor_scalar(
                        out=exm[:, P:2 * P], in0=iota_row[:],
                        scalar1=dpjt[:, dpos:dpos + 1],
                        scalar2=ex2[:, k, 1:2], op0=op.is_equal, op1=op.mult)
                    nc.tensor.matmul(out=gtt[:], lhsT=hg[:, col, 0:64].bitcast(dt.bfloat16),
                                     rhs=exm[:], start=st_, stop=sp_)
                    nc.tensor.matmul(out=ss0[:], lhsT=exm[:, 0:P], rhs=ones_col[:],
                                     start=st_, stop=sp_)
                    nc.tensor.matmul(out=ss1[:], lhsT=exm[:, P:2 * P], rhs=ones_col[:],
                                     start=st_, stop=sp_)

                # ---- finalize block b
                rec = fin.tile([P, 2], dt.float32, tag="rec")
                nc.vector.reciprocal(out=rec[:, 0:1], in_=ss0[:])
                nc.vector.reciprocal(out=rec[:, 1:2], in_=ss1[:])
                ob = fin.tile([P, HEADS * C], dt.float32, tag="ob")
                for hd in range(HEADS):
                    gs = fin.tile([P, P], dt.bfloat16, tag="gs")
                    nc.scalar.activation(out=gs[:], in_=gtt[:, hd * P:(hd + 1) * P],
                                         func=act.Copy)
                    u = psU.tile([P, C], dt.float32, tag="u", space="PSUM")
                    nc.tensor.matmul(out=u[:], lhsT=gs[:],
                                     rhs=wsb[:, hd * C:(hd + 1) * C],
                                     start=True, stop=True)
                    o2 = fin.tile([P, C], dt.float32, tag="o2")
                    nc.vector.tensor_scalar(out=o2[:], in0=u[:],
                                            scalar1=rec[:, hd:hd + 1],
                                            scalar2=None, op0=op.mult)
                    o3 = fin.tile([P, C], dt.float32, tag="o3")
                    nc.vector.tensor_tensor(out=o3[:], in0=o2[:],
                                            in1=bias_bc[:, hd * C:(hd + 1) * C],
                                            op=op.add)
                    a1 = fin.tile([P, C], dt.float32, tag="a1")
                    nc.vector.tensor_scalar(out=a1[:], in0=o3[:], scalar1=0.0,
                                            scalar2=None, op0=op.min)
                    e1 = fin.tile([P, C], dt.float32, tag="e1")
                    nc.scalar.activation(out=e1[:], in_=a1[:], func=act.Exp)
                    a3 = fin.tile([P, C], dt.float32, tag="a3")
                    nc.vector.tensor_scalar(out=a3[:], in0=o3[:], scalar1=0.0,
                                            scalar2=-1.0, op0=op.max, op1=op.add)
                    nc.vector.tensor_tensor(out=ob[:, hd * C:(hd + 1) * C],
                                            in0=a3[:], in1=e1[:], op=op.add)
                nc.sync.dma_start(out_t.ap()[b * P:(b + 1) * P, :], ob[:])

    nc.compile()
    return nc


def _get_program(params):
    if params not in _CACHE:
        _CACHE[params] = _build(params)
    return _CACHE[params]


# ------------------------------------------------------------------- kernel
def kernel(h_node, edge_index, W, att_src, att_dst, bias):
    from concourse.bass_utils import run_bass_kernel_spmd

    h_node = np.asarray(h_node, dtype=np.float32)
    W = np.asarray(W, dtype=np.float32)
    att_src = np.asarray(att_src, dtype=np.float32)
    att_dst = np.asarray(att_dst, dtype=np.float32)
    bias = np.asarray(bias, dtype=np.float32)

    params, wlo, whi, dpjL_t, dpjH_t, djpL, djpH = _prep(np.asarray(edge_index))
    hpack, hT, h_shT, wa4, wsb, bias2 = _pack_inputs(h_node, W, att_src, att_dst, bias)
    nc = _get_program(params)

    KLs, KHs, _ = params
    NL = sum(KLs)
    NH = sum(KHs)
    in_maps = []
    for c in range(NC_CORES):
        in_maps.append({
            "hpack": hpack, "hT": hT, "hshT": h_shT[c], "wa4": wa4,
            "wsb": wsb, "bias_in": bias2,
            "wlo": wlo[c], "whi": whi[c],
            "dpjL": dpjL_t[c], "dpjH": dpjH_t[c],
            "djpL": djpL[c].reshape(1, NL * 128),
            "djpH": djpH[c].reshape(1, NH * 128),
        })
    res = run_bass_kernel_spmd(nc, in_maps, core_ids=list(range(NC_CORES)))
    out = np.concatenate([res.results[c]["out"][:SH] for c in range(NC_CORES)], axis=0)
    return out


# revision 13
# speedup vs baseline: 2.8513x; 1.4570x over previous
"""GAT layer (PyG GATConv eval, 2 heads x 128, self-loops, ELU) on 8 trn2 cores.

v2 strategy (dst-sharded, per core):
  hpack[N,128] f32 rows (512B): cols 0:64 = h as packed bf16 pairs (host),
  cols 64:66 = a_src logits f32 (device phase A embeds).  ONE dma_gather by
  src id per edge slot fetches h (bf16) + a_src together.
  Phase A: a4 = hT_tile^T @ wa4 (host passes hT bf16, no PE transposes),
  embed a_src into hpack; local-shard a_dst kept in SBUF (h_shT input).
  Phase B: edges grouped by dst block (128 dsts), chunks of 128 slots,
  per-block chunk counts = max over cores (not global max).  Per chunk:
  exm = fused is_equal+mult masks (bf16), gtt += hg^T @ exm (bf16 PE),
  segsum via exm^T @ ones, a_dst per slot via mask-transpose matmuls.
  Finalize: U = GT^T W (bf16), normalize, +bias, exact ELU.
"""
import math
from contextlib import ExitStack

import numpy as np
import ml_dtypes

BF16 = ml_dtypes.bfloat16
HEADS = 2
C = 128
IN = 128
N = 50000
NC_CORES = 8
SH = N // NC_CORES            # 6250 dst nodes per core
NBLK = math.ceil(SH / 128)    # 49 dst blocks per core
SHP = NBLK * 128              # padded shard rows 6272
NTILE = math.ceil(N / 128)    # 391 tiles of full h
NPAD = NTILE * 128            # 50048 padded rows of hpack
LO = 32768                    # int16 gather index split
RL = 40                       # max lo chunks per gather run
RH = 24                       # max hi chunks per gather run
NEG_SLOPE = 0.2

_CACHE = {}


# ----------------------------------------------------------------- host prep
def _wrap16(idx, nchunk):
    """idx [nchunk*128] int16 -> wrapped gather table [128, nchunk*8]."""
    sl = idx.reshape(nchunk * 8, 16)            # [col, p16]
    w = np.broadcast_to(sl.T[None, :, :], (8, 16, nchunk * 8))
    return np.ascontiguousarray(w.reshape(128, nchunk * 8))


def _prep(edge_index):
    src = np.concatenate([edge_index[0], np.arange(N, dtype=np.int64)])
    dst = np.concatenate([edge_index[1], np.arange(N, dtype=np.int64)])
    core = dst // SH
    blk = (dst % SH) // 128
    dloc = (dst % SH) % 128
    half = (src >= LO).astype(np.int64)

    # per (core, block, half) counts -> per-block chunk counts (max over cores)
    cnt = np.zeros((NC_CORES, NBLK, 2), dtype=np.int64)
    np.add.at(cnt, (core, blk, half), 1)
    KL = np.maximum(np.ceil(cnt[:, :, 0] / 128).astype(np.int64).max(0), 0)
    KH = np.maximum(np.ceil(cnt[:, :, 1] / 128).astype(np.int64).max(0), 0)
    offL = np.concatenate([[0], np.cumsum(KL)])   # lo-stream chunk offsets
    offH = np.concatenate([[0], np.cumsum(KH)])
    NL, NH = int(offL[-1]), int(offH[-1])

    # slot assignment: stable sort by (core, blk, half); rank within group
    key = (core * NBLK + blk) * 2 + half
    order = np.argsort(key, kind="stable")
    key_s = key[order]
    sizes = np.bincount(key_s, minlength=NC_CORES * NBLK * 2)
    starts = np.concatenate([[0], np.cumsum(sizes)[:-1]])
    rank = np.arange(len(key_s)) - starts[key_s]
    src_s = src[order]
    dloc_s = dloc[order]
    core_s = key_s // (2 * NBLK)
    blk_s = (key_s // 2) % NBLK
    half_s = key_s % 2

    # global slot position within each core's lo/hi stream
    strm_off = np.where(half_s == 0, offL[blk_s] * 128, offH[blk_s] * 128)
    slot = strm_off + rank

    idxL = np.zeros((NC_CORES, NL * 128), dtype=np.int16)
    idxH = np.zeros((NC_CORES, NH * 128), dtype=np.int16)
    dpjL = np.full((NC_CORES, NL, 128), 999.0, dtype=np.float32)
    dpjH = np.full((NC_CORES, NH, 128), 999.0, dtype=np.float32)

    lo_m = half_s == 0
    idxL[core_s[lo_m], slot[lo_m]] = src_s[lo_m].astype(np.int16)
    idxH[core_s[~lo_m], slot[~lo_m]] = (src_s[~lo_m] - LO).astype(np.int16)
    dpjL[core_s[lo_m], slot[lo_m] // 128, slot[lo_m] % 128] = dloc_s[lo_m]
    dpjH[core_s[~lo_m], slot[~lo_m] // 128, slot[~lo_m] % 128] = dloc_s[~lo_m]

    wlo = np.stack([_wrap16(idxL[c], NL) for c in range(NC_CORES)])
    whi = np.stack([_wrap16(idxH[c], NH) for c in range(NC_CORES)])
    # dpj tables [128 partitions, nchunk] (scalar per partition per chunk)
    dpjL_t = np.ascontiguousarray(dpjL.transpose(0, 2, 1))
    dpjH_t = np.ascontiguousarray(dpjH.transpose(0, 2, 1))
    # djp rows [1, nchunk*128] bf16 for the PE broadcast matmul
    djpL = dpjL.reshape(NC_CORES, NL * 128).astype(BF16)
    djpH = dpjH.reshape(NC_CORES, NH * 128).astype(BF16)

    # gather runs: greedy whole blocks with sum KL<=RL and sum KH<=RH
    runs = []
    b = 0
    while b < NBLK:
        b1 = b + 1
        while b1 < NBLK and (KL[b:b1 + 1].sum() <= RL and KH[b:b1 + 1].sum() <= RH):
            b1 += 1
        runs.append((b, b1))
        b = b1
    params = (tuple(int(k) for k in KL), tuple(int(k) for k in KH),
              tuple(runs))
    return params, wlo, whi, dpjL_t, dpjH_t, djpL, djpH


def _pack_inputs(h_node, W, att_src, att_dst, bias):
    hb = h_node.astype(BF16)                       # [N,128] bf16
    hpack = np.zeros((NPAD, 128), dtype=np.float32)
    hpack[:N, 0:64] = hb.view(np.uint16).reshape(N, 64, 2).view(np.uint32).reshape(N, 64).view(np.float32)
    hT = np.zeros((128, NPAD), dtype=BF16)
    hT[:, :N] = hb.T
    h_shT = np.zeros((NC_CORES, 128, SHP), dtype=BF16)
    for c in range(NC_CORES):
        h_shT[c, :, :SH] = hb[c * SH:(c + 1) * SH].T
    W3 = W.reshape(IN, HEADS, C)
    wa4 = np.stack([
        np.einsum('cho,ho->c', W3, att_src * (np.arange(HEADS)[:, None] == 0)),
        np.einsum('cho,ho->c', W3, att_src * (np.arange(HEADS)[:, None] == 1)),
        np.einsum('cho,ho->c', W3, att_dst * (np.arange(HEADS)[:, None] == 0)),
        np.einsum('cho,ho->c', W3, att_dst * (np.arange(HEADS)[:, None] == 1)),
    ], axis=1).astype(BF16)                        # [128, 4]
    wsb = W.astype(BF16)                           # [128, 256]
    bias2 = bias.reshape(1, HEADS * C).astype(np.float32)
    return hpack, hT, h_shT, wa4, wsb, bias2


# ------------------------------------------------------------ device program
def _build(params):
    import concourse.bacc as bacc
    import concourse.bass as bass
    import concourse.mybir as mybir
    import concourse.tile as tile

    KL, KH, runs = params
    offL = [0]
    for k in KL:
        offL.append(offL[-1] + k)
    offH = [0]
    for k in KH:
        offH.append(offH[-1] + k)
    NL, NH = offL[-1], offH[-1]
    KMAX = max(KL[b] + KH[b] for b in range(NBLK))

    dt = mybir.dt
    op = mybir.AluOpType
    act = mybir.ActivationFunctionType
    P = 128

    nc = bacc.Bacc("TRN2", target_bir_lowering=False, debug=False,
                   num_devices=NC_CORES)
    hpack = nc.dram_tensor("hpack", [NPAD, 128], dt.float32, kind="ExternalInput")
    hT_in = nc.dram_tensor("hT", [128, NPAD], dt.bfloat16, kind="ExternalInput")
    hshT_in = nc.dram_tensor("hshT", [128, SHP], dt.bfloat16, kind="ExternalInput")
    wa4_in = nc.dram_tensor("wa4", [128, 4], dt.bfloat16, kind="ExternalInput")
    wsb_in = nc.dram_tensor("wsb", [128, HEADS * C], dt.bfloat16, kind="ExternalInput")
    bias_in = nc.dram_tensor("bias_in", [1, HEADS * C], dt.float32, kind="ExternalInput")
    wlo_in = nc.dram_tensor("wlo", [P, max(NL, 1) * 8], dt.int16, kind="ExternalInput")
    whi_in = nc.dram_tensor("whi", [P, max(NH, 1) * 8], dt.int16, kind="ExternalInput")
    dpjL_in = nc.dram_tensor("dpjL", [P, max(NL, 1)], dt.float32, kind="ExternalInput")
    dpjH_in = nc.dram_tensor("dpjH", [P, max(NH, 1)], dt.float32, kind="ExternalInput")
    djpL_in = nc.dram_tensor("djpL", [1, max(NL, 1) * 128], dt.bfloat16, kind="ExternalInput")
    djpH_in = nc.dram_tensor("djpH", [1, max(NH, 1) * 128], dt.bfloat16, kind="ExternalInput")
    out_t = nc.dram_tensor("out", [SHP, HEADS * C], dt.float32, kind="ExternalOutput")

    with tile.TileContext(nc) as tc, ExitStack() as ctx:
        const = ctx.enter_context(tc.tile_pool(name="const", bufs=1))

        # ---- constants
        iota_row_f = const.tile([P, P], dt.float32)
        nc.gpsimd.iota(iota_row_f[:], pattern=[[1, P]], base=0,
                       channel_multiplier=0, allow_small_or_imprecise_dtypes=True)
        iota_row = const.tile([P, P], dt.bfloat16)
        nc.vector.tensor_copy(out=iota_row[:], in_=iota_row_f[:])
        iota_col4 = const.tile([P, 1024], dt.float32)
        nc.gpsimd.iota(iota_col4[:], pattern=[[0, 1024]], base=0,
                       channel_multiplier=1, allow_small_or_imprecise_dtypes=True)
        iota_colb = const.tile([P, 1024], dt.bfloat16)
        nc.vector.tensor_copy(out=iota_colb[:], in_=iota_col4[:])
        ones1 = const.tile([1, P], dt.bfloat16)
        nc.gpsimd.memset(ones1[:], 1.0)
        ones_col = const.tile([P, 1], dt.bfloat16)
        nc.gpsimd.memset(ones_col[:], 1.0)
        wa4_sb = const.tile([P, 4], dt.bfloat16)
        nc.sync.dma_start(wa4_sb[:], wa4_in.ap()[:, :])
        wsb = const.tile([P, HEADS * C], dt.bfloat16)
        nc.sync.dma_start(wsb[:], wsb_in.ap()[:, :])
        bias_bc = const.tile([P, HEADS * C], dt.float32)
        nc.sync.dma_start(bias_bc[:], bass.AP(bias_in, 0, [[0, P], [1, HEADS * C]]))
        adst_sb = const.tile([P, NBLK, 2], dt.float32)
        adst_bf = const.tile([P, NBLK, 2], dt.bfloat16)

        # ---- phase B input tables (preload during phase A)
        wloT = const.tile([P, max(NL, 1) * 8], dt.int16)
        nc.sync.dma_start(wloT[:], wlo_in.ap()[:, :])
        whiT = const.tile([P, max(NH, 1) * 8], dt.int16)
        nc.sync.dma_start(whiT[:], whi_in.ap()[:, :])
        dpjL_sb = const.tile([P, max(NL, 1)], dt.float32)
        nc.sync.dma_start(dpjL_sb[:], dpjL_in.ap()[:, :])
        dpjH_sb = const.tile([P, max(NH, 1)], dt.float32)
        nc.sync.dma_start(dpjH_sb[:], dpjH_in.ap()[:, :])

        # ---- phase A: a4 = hT_tile^T @ wa4 for all N; embed a_src into hpack
        ctxA = ExitStack()
        sbA = ctxA.enter_context(tc.tile_pool(name="sbA", bufs=3))
        psA = ctxA.enter_context(tc.tile_pool(name="psA", bufs=3, space="PSUM"))
        st = const.tile([P, NTILE, 4], dt.float32)   # a4 staging, all tiles
        GA = 96                     # h tiles per hT DMA
        t = 0
        while t < NTILE:
            nt = min(GA, NTILE - t)
            ht = sbA.tile([P, GA * 128], dt.bfloat16, tag="ht")
            nc.sync.dma_start(ht[:, :nt * 128],
                              hT_in.ap()[:, t * 128:(t + nt) * 128])
            for g0 in range(0, nt, 4):
                n4 = min(4, nt - g0)
                a4p = psA.tile([P, 4, 4], dt.float32, tag="a4", space="PSUM")
                for g in range(n4):
                    nc.tensor.matmul(out=a4p[:, g, :],
                                     lhsT=ht[:, (g0 + g) * 128:(g0 + g + 1) * 128],
                                     rhs=wa4_sb[:], start=True, stop=True)
                nc.scalar.activation(out=st[:, t + g0:t + g0 + n4, :],
                                      in_=a4p[:, :n4, :], func=act.Copy)
            t += nt
        # embed a_src (cols 0:2 of a4) into hpack cols 64:66
        emb_ap = bass.AP(hpack, 64, [[128, P], [128 * 128, NTILE], [1, 2]])
        nc.sync.dma_start(emb_ap, st[:, :, 0:2])

        # phase A-bis: local shard a_dst from h_shT
        hts = sbA.tile([P, SHP], dt.bfloat16, tag="hts")
        nc.sync.dma_start(hts[:], hshT_in.ap()[:, :])
        for t4 in range(13):
            nt = min(4, NBLK - t4 * 4)
            if nt <= 0:
                break
            a4p = psA.tile([P, 4, 2], dt.float32, tag="a4b", space="PSUM")
            for g in range(nt):
                nc.tensor.matmul(out=a4p[:, g, :],
                                 lhsT=hts[:, (t4 * 4 + g) * 128:(t4 * 4 + g + 1) * 128],
                                 rhs=wa4_sb[:, 2:4], start=True, stop=True)
            nc.scalar.activation(out=adst_sb[:, t4 * 4:t4 * 4 + nt, :],
                                  in_=a4p[:, :nt, :], func=act.Copy)
        nc.vector.tensor_copy(out=adst_bf[:], in_=adst_sb[:])
        ctxA.close()

        # ---- phase B
        ghL = ctx.enter_context(tc.tile_pool(name="ghL", bufs=2))
        ghH = ctx.enter_context(tc.tile_pool(name="ghH", bufs=2))
        gdj = ctx.enter_context(tc.tile_pool(name="gdj", bufs=2))
        mk = ctx.enter_context(tc.tile_pool(name="mk", bufs=3))
        sm = ctx.enter_context(tc.tile_pool(name="sm", bufs=2))
        fin = ctx.enter_context(tc.tile_pool(name="fin", bufs=2))
        psGT = ctx.enter_context(tc.tile_pool(name="psGT", bufs=2, space="PSUM"))
        psAD = ctx.enter_context(tc.tile_pool(name="psAD", bufs=2, space="PSUM"))
        psSS = ctx.enter_context(tc.tile_pool(name="psSS", bufs=1, space="PSUM"))
        psU = ctx.enter_context(tc.tile_pool(name="psU", bufs=1, space="PSUM"))

        hp_ap = hpack.ap()
        for (b0, b1) in runs:
            nL = offL[b1] - offL[b0]
            nH = offH[b1] - offH[b0]
            hgl = ghL.tile([P, RL, 128], dt.float32, tag="hgl")
            if nL:
                nc.gpsimd.dma_gather(
                    out_ap=hgl[:, :nL, :], in_ap=hp_ap[0:LO, :],
                    idxs_ap=wloT[:, offL[b0] * 8:offL[b1] * 8],
                    num_idxs=nL * P, num_idxs_reg=nL * P,
                    elem_size=128, single_packet=False)
            hgh = ghH.tile([P, RH, 128], dt.float32, tag="hgh")
            if nH:
                nc.gpsimd.dma_gather(
                    out_ap=hgh[:, :nH, :], in_ap=hp_ap[LO:NPAD, :],
                    idxs_ap=whiT[:, offH[b0] * 8:offH[b1] * 8],
                    num_idxs=nH * P, num_idxs_reg=nH * P,
                    elem_size=128, single_packet=False)
            djl = gdj.tile([1, RL * 128], dt.bfloat16, tag="djl")
            if nL:
                nc.sync.dma_start(djl[:, :nL * 128],
                                  djpL_in.ap()[:, offL[b0] * 128:offL[b1] * 128])
            djh = gdj.tile([1, RH * 128], dt.bfloat16, tag="djh")
            if nH:
                nc.sync.dma_start(djh[:, :nH * 128],
                                  djpH_in.ap()[:, offH[b0] * 128:offH[b1] * 128])

            for b in range(b0, b1):
                kl, kh = KL[b], KH[b]
                K = kl + kh
                if K == 0:
                    continue
                # chunk descriptors: (hg tile, col in tile, dpj table, pos)
                chunks = []
                for j in range(kl):
                    chunks.append((hgl, offL[b] - offL[b0] + j, dpjL_sb,
                                   offL[b] + j, djl, (offL[b] - offL[b0] + j)))
                for j in range(kh):
                    chunks.append((hgh, offH[b] - offH[b0] + j, dpjH_sb,
                                   offH[b] + j, djh, (offH[b] - offH[b0] + j)))

                # pass 1: a_dst per slot via mask-transpose matmuls
                adp = psAD.tile([P, KMAX, 2], dt.float32, tag="adp", space="PSUM")
                for g0 in range(0, K, 8):
                    ng = min(8, K - g0)
                    dbc = mk.tile([P, 8 * 128], dt.bfloat16, tag="dbc")
                    # pbc needs contiguous djl cols: lo and hi parts separate
                    done = 0
                    while done < ng:
                        djt, dcol = chunks[g0 + done][4], chunks[g0 + done][5]
                        nrun = 1
                        while (done + nrun < ng
                               and chunks[g0 + done + nrun][4] is djt
                               and chunks[g0 + done + nrun][5] == dcol + nrun):
                            nrun += 1
                        nc.gpsimd.partition_broadcast(
                            dbc[:, done * 128:(done + nrun) * 128],
                            djt[0:1, dcol * 128:(dcol + nrun) * 128])
                        done += nrun
                    mt4 = mk.tile([P, 8 * 128], dt.bfloat16, tag="mt4")
                    nc.vector.tensor_tensor(
                        out=mt4[:, :ng * 128], in0=iota_colb[:, :ng * 128],
                        in1=dbc[:, :ng * 128], op=op.is_equal)
                    for gg in range(ng):
                        nc.tensor.matmul(
                            out=adp[:, g0 + gg, :],
                            lhsT=mt4[:, gg * 128:(gg + 1) * 128],
                            rhs=adst_bf[:, b, :], start=True, stop=True)

                # logits -> ex  [P, K, 2]
                tsum = sm.tile([P, KMAX, 2], dt.float32, tag="tsum")
                if kl:
                    nc.vector.tensor_tensor(
                        out=tsum[:, :kl, :],
                        in0=hgl[:, offL[b] - offL[b0]:offL[b] - offL[b0] + kl, 64:66],
                        in1=adp[:, :kl, :], op=op.add)
                if kh:
                    nc.vector.tensor_tensor(
                        out=tsum[:, kl:K, :],
                        in0=hgh[:, offH[b] - offH[b0]:offH[b] - offH[b0] + kh, 64:66],
                        in1=adp[:, kl:K, :], op=op.add)
                u02 = sm.tile([P, KMAX, 2], dt.float32, tag="u02")
                nc.vector.tensor_scalar(out=u02[:, :K, :], in0=tsum[:, :K, :],
                                        scalar1=NEG_SLOPE, scalar2=None, op0=op.mult)
                lr = sm.tile([P, KMAX, 2], dt.float32, tag="lr")
                nc.vector.tensor_tensor(out=lr[:, :K, :], in0=tsum[:, :K, :],
                                        in1=u02[:, :K, :], op=op.max)
                ex2 = sm.tile([P, KMAX, 2], dt.float32, tag="ex2")
                nc.scalar.activation(out=ex2[:, :K, :], in_=lr[:, :K, :], func=act.Exp)

                # pass 2: masked scatter matmuls
                gtt = psGT.tile([P, HEADS * P], dt.float32, tag="gtt", space="PSUM")
                ss0 = psSS.tile([P, 1], dt.float32, tag="ss0", space="PSUM")
                ss1 = psSS.tile([P, 1], dt.float32, tag="ss1", space="PSUM")
                for k, (hg, col, dpjt, dpos, _, _) in enumerate(chunks):
                    st_, sp_ = k == 0, k == K - 1
                    exm = mk.tile([P, 2 * P], dt.bfloat16, tag="exm")
                    nc.vector.tensor_scalar(
                        out=exm[:, 0:P], in0=iota_row[:],
                        scalar1=dpjt[:, dpos:dpos + 1],
                        scalar2=ex2[:, k, 0:1], op0=op.is_equal, op1=op.mult)
                    nc.vector.tensor_scalar(
                        out=exm[:, P:2 * P], in0=iota_row[:],
                        scalar1=dpjt[:, dpos:dpos + 1],
                        scalar2=ex2[:, k, 1:2], op0=op.is_equal, op1=op.mult)
                    nc.tensor.matmul(out=gtt[:], lhsT=hg[:, col, 0:64].bitcast(dt.bfloat16),
                                     rhs=exm[:], start=st_, stop=sp_)
                    nc.tensor.matmul(out=ss0[:], lhsT=exm[:, 0:P], rhs=ones_col[:],
                                     start=st_, stop=sp_)
                    nc.tensor.matmul(out=ss1[:], lhsT=exm[:, P:2 * P], rhs=ones_col[:],
                                     start=st_, stop=sp_)

                # ---- finalize block b
                rec = fin.tile([P, 2], dt.float32, tag="rec")
                nc.vector.reciprocal(out=rec[:, 0:1], in_=ss0[:])
                nc.vector.reciprocal(out=rec[:, 1:2], in_=ss1[:])
                ob = fin.tile([P, HEADS * C], dt.float32, tag="ob")
                for hd in range(HEADS):
                    gs = fin.tile([P, P], dt.bfloat16, tag="gs")
                    nc.scalar.activation(out=gs[:], in_=gtt[:, hd * P:(hd + 1) * P],
                                         func=act.Copy)
                    u = psU.tile([P, C], dt.float32, tag="u", space="PSUM")
                    nc.tensor.matmul(out=u[:], lhsT=gs[:],
                                     rhs=wsb[:, hd * C:(hd + 1) * C],
                                     start=True, stop=True)
                    o2 = fin.tile([P, C], dt.float32, tag="o2")
                    nc.scalar.activation(out=o2[:], in_=u[:], func=act.Copy,
                                         scale=rec[:, hd:hd + 1])
                    o3 = fin.tile([P, C], dt.float32, tag="o3")
                    nc.vector.tensor_tensor(out=o3[:], in0=o2[:],
                                            in1=bias_bc[:, hd * C:(hd + 1) * C],
                                            op=op.add)
                    rl = fin.tile([P, C], dt.float32, tag="rl")
                    nc.scalar.activation(out=rl[:], in_=o3[:], func=act.Relu,
                                         scale=-1.0)
                    e1 = fin.tile([P, C], dt.float32, tag="e1")
                    nc.scalar.activation(out=e1[:], in_=rl[:], func=act.Exp,
                                         scale=-1.0)
                    a3 = fin.tile([P, C], dt.float32, tag="a3")
                    nc.vector.tensor_scalar(out=a3[:], in0=o3[:], scalar1=0.0,
                                            scalar2=-1.0, op0=op.max, op1=op.add)
                    nc.vector.tensor_tensor(out=ob[:, hd * C:(hd + 1) * C],
                                            in0=a3[:], in1=e1[:], op=op.add)
                nc.sync.dma_start(out_t.ap()[b * P:(b + 1) * P, :], ob[:])

    nc.compile()
    return nc


def _get_program(params):
    if params not in _CACHE:
        _CACHE[params] = _build(params)
    return _CACHE[params]


# ------------------------------------------------------------------- kernel
def kernel(h_node, edge_index, W, att_src, att_dst, bias):
    from concourse.bass_utils import run_bass_kernel_spmd

    h_node = np.asarray(h_node, dtype=np.float32)
    W = np.asarray(W, dtype=np.float32)
    att_src = np.asarray(att_src, dtype=np.float32)
    att_dst = np.asarray(att_dst, dtype=np.float32)
    bias = np.asarray(bias, dtype=np.float32)

    params, wlo, whi, dpjL_t, dpjH_t, djpL, djpH = _prep(np.asarray(edge_index))
    hpack, hT, h_shT, wa4, wsb, bias2 = _pack_inputs(h_node, W, att_src, att_dst, bias)
    nc = _get_program(params)

    KLs, KHs, _ = params
    NL = sum(KLs)
    NH = sum(KHs)
    in_maps = []
    for c in range(NC_CORES):
        in_maps.append({
            "hpack": hpack, "hT": hT, "hshT": h_shT[c], "wa4": wa4,
            "wsb": wsb, "bias_in": bias2,
            "wlo": wlo[c], "whi": whi[c],
            "dpjL": dpjL_t[c], "dpjH": dpjH_t[c],
            "djpL": djpL[c].reshape(1, NL * 128),
            "djpH": djpH[c].reshape(1, NH * 128),
        })
    res = run_bass_kernel_spmd(nc, in_maps, core_ids=list(range(NC_CORES)))
    out = np.concatenate([res.results[c]["out"][:SH] for c in range(NC_CORES)], axis=0)
    return out


# revision 15
# speedup vs baseline: 2.8930x; 1.0146x over previous
"""GAT layer (PyG GATConv eval, 2 heads x 128, self-loops, ELU) on 8 trn2 cores.

v2 strategy (dst-sharded, per core):
  hpack[N,128] f32 rows (512B): cols 0:64 = h as packed bf16 pairs (host),
  cols 64:66 = a_src logits f32 (device phase A embeds).  ONE dma_gather by
  src id per edge slot fetches h (bf16) + a_src together.
  Phase A: a4 = hT_tile^T @ wa4 (host passes hT bf16, no PE transposes),
  embed a_src into hpack; local-shard a_dst kept in SBUF (h_shT input).
  Phase B: edges grouped by dst block (128 dsts), chunks of 128 slots,
  per-block chunk counts = max over cores (not global max).  Per chunk:
  exm = fused is_equal+mult masks (bf16), gtt += hg^T @ exm (bf16 PE),
  segsum via exm^T @ ones, a_dst per slot via mask-transpose matmuls.
  Finalize: U = GT^T W (bf16), normalize, +bias, exact ELU.
"""
import math
from contextlib import ExitStack

import numpy as np
import ml_dtypes

BF16 = ml_dtypes.bfloat16
HEADS = 2
C = 128
IN = 128
N = 50000
NC_CORES = 8
SH = N // NC_CORES            # 6250 dst nodes per core
NBLK = math.ceil(SH / 128)    # 49 dst blocks per core
SHP = NBLK * 128              # padded shard rows 6272
NTILE = math.ceil(N / 128)    # 391 tiles of full h
NPAD = NTILE * 128            # 50048 padded rows of hpack
LO = 32768                    # int16 gather index split
RL = 40                       # max lo chunks per gather run
RH = 24                       # max hi chunks per gather run
NEG_SLOPE = 0.2

_CACHE = {}


# ----------------------------------------------------------------- host prep
def _wrap16(idx, nchunk):
    """idx [nchunk*128] int16 -> wrapped gather table [128, nchunk*8]."""
    sl = idx.reshape(nchunk * 8, 16)            # [col, p16]
    w = np.broadcast_to(sl.T[None, :, :], (8, 16, nchunk * 8))
    return np.ascontiguousarray(w.reshape(128, nchunk * 8))


def _prep(edge_index):
    src = np.concatenate([edge_index[0], np.arange(N, dtype=np.int64)])
    dst = np.concatenate([edge_index[1], np.arange(N, dtype=np.int64)])
    core = dst // SH
    blk = (dst % SH) // 128
    dloc = (dst % SH) % 128
    half = (src >= LO).astype(np.int64)

    # per (core, block, half) counts -> per-block chunk counts (max over cores)
    cnt = np.zeros((NC_CORES, NBLK, 2), dtype=np.int64)
    np.add.at(cnt, (core, blk, half), 1)
    KL = np.maximum(np.ceil(cnt[:, :, 0] / 128).astype(np.int64).max(0), 0)
    KH = np.maximum(np.ceil(cnt[:, :, 1] / 128).astype(np.int64).max(0), 0)
    offL = np.concatenate([[0], np.cumsum(KL)])   # lo-stream chunk offsets
    offH = np.concatenate([[0], np.cumsum(KH)])
    NL, NH = int(offL[-1]), int(offH[-1])

    # slot assignment: stable sort by (core, blk, half); rank within group
    key = (core * NBLK + blk) * 2 + half
    order = np.argsort(key, kind="stable")
    key_s = key[order]
    sizes = np.bincount(key_s, minlength=NC_CORES * NBLK * 2)
    starts = np.concatenate([[0], np.cumsum(sizes)[:-1]])
    rank = np.arange(len(key_s)) - starts[key_s]
    src_s = src[order]
    dloc_s = dloc[order]
    core_s = key_s // (2 * NBLK)
    blk_s = (key_s // 2) % NBLK
    half_s = key_s % 2

    # global slot position within each core's lo/hi stream
    strm_off = np.where(half_s == 0, offL[blk_s] * 128, offH[blk_s] * 128)
    slot = strm_off + rank

    idxL = np.zeros((NC_CORES, NL * 128), dtype=np.int16)
    idxH = np.zeros((NC_CORES, NH * 128), dtype=np.int16)
    dpjL = np.full((NC_CORES, NL, 128), 999.0, dtype=np.float32)
    dpjH = np.full((NC_CORES, NH, 128), 999.0, dtype=np.float32)

    lo_m = half_s == 0
    idxL[core_s[lo_m], slot[lo_m]] = src_s[lo_m].astype(np.int16)
    idxH[core_s[~lo_m], slot[~lo_m]] = (src_s[~lo_m] - LO).astype(np.int16)
    dpjL[core_s[lo_m], slot[lo_m] // 128, slot[lo_m] % 128] = dloc_s[lo_m]
    dpjH[core_s[~lo_m], slot[~lo_m] // 128, slot[~lo_m] % 128] = dloc_s[~lo_m]

    wlo = np.stack([_wrap16(idxL[c], NL) for c in range(NC_CORES)])
    whi = np.stack([_wrap16(idxH[c], NH) for c in range(NC_CORES)])
    # dpj tables [128 partitions, nchunk] (scalar per partition per chunk)
    dpjL_t = np.ascontiguousarray(dpjL.transpose(0, 2, 1))
    dpjH_t = np.ascontiguousarray(dpjH.transpose(0, 2, 1))
    # djp rows [1, nchunk*128] bf16 for the PE broadcast matmul
    djpL = dpjL.reshape(NC_CORES, NL * 128).astype(BF16)
    djpH = dpjH.reshape(NC_CORES, NH * 128).astype(BF16)

    # gather runs: greedy whole blocks with sum KL<=RL and sum KH<=RH
    runs = []
    b = 0
    while b < NBLK:
        b1 = b + 1
        while b1 < NBLK and (KL[b:b1 + 1].sum() <= RL and KH[b:b1 + 1].sum() <= RH):
            b1 += 1
        runs.append((b, b1))
        b = b1
    params = (tuple(int(k) for k in KL), tuple(int(k) for k in KH),
              tuple(runs))
    return params, wlo, whi, dpjL_t, dpjH_t, djpL, djpH


def _pack_inputs(h_node, W, att_src, att_dst, bias):
    hb = h_node.astype(BF16)                       # [N,128] bf16
    hpack = np.zeros((NPAD, 128), dtype=np.float32)
    hpack[:N, 0:64] = hb.view(np.uint16).reshape(N, 64, 2).view(np.uint32).reshape(N, 64).view(np.float32)
    hT = np.zeros((128, NPAD), dtype=BF16)
    hT[:, :N] = hb.T
    h_shT = np.zeros((NC_CORES, 128, SHP), dtype=BF16)
    for c in range(NC_CORES):
        h_shT[c, :, :SH] = hb[c * SH:(c + 1) * SH].T
    W3 = W.reshape(IN, HEADS, C)
    wa4 = np.stack([
        np.einsum('cho,ho->c', W3, att_src * (np.arange(HEADS)[:, None] == 0)),
        np.einsum('cho,ho->c', W3, att_src * (np.arange(HEADS)[:, None] == 1)),
        np.einsum('cho,ho->c', W3, att_dst * (np.arange(HEADS)[:, None] == 0)),
        np.einsum('cho,ho->c', W3, att_dst * (np.arange(HEADS)[:, None] == 1)),
    ], axis=1).astype(BF16)                        # [128, 4]
    wsb = W.astype(BF16)                           # [128, 256]
    bias2 = bias.reshape(1, HEADS * C).astype(np.float32)
    return hpack, hT, h_shT, wa4, wsb, bias2


# ------------------------------------------------------------ device program
def _build(params):
    import concourse.bacc as bacc
    import concourse.bass as bass
    import concourse.mybir as mybir
    import concourse.tile as tile

    KL, KH, runs = params
    offL = [0]
    for k in KL:
        offL.append(offL[-1] + k)
    offH = [0]
    for k in KH:
        offH.append(offH[-1] + k)
    NL, NH = offL[-1], offH[-1]
    KMAX = max(KL[b] + KH[b] for b in range(NBLK))

    dt = mybir.dt
    op = mybir.AluOpType
    act = mybir.ActivationFunctionType
    P = 128

    nc = bacc.Bacc("TRN2", target_bir_lowering=False, debug=False,
                   num_devices=NC_CORES)
    hpack = nc.dram_tensor("hpack", [NPAD, 128], dt.float32, kind="ExternalInput")
    hT_in = nc.dram_tensor("hT", [128, NPAD], dt.bfloat16, kind="ExternalInput")
    hshT_in = nc.dram_tensor("hshT", [128, SHP], dt.bfloat16, kind="ExternalInput")
    wa4_in = nc.dram_tensor("wa4", [128, 4], dt.bfloat16, kind="ExternalInput")
    wsb_in = nc.dram_tensor("wsb", [128, HEADS * C], dt.bfloat16, kind="ExternalInput")
    bias_in = nc.dram_tensor("bias_in", [1, HEADS * C], dt.float32, kind="ExternalInput")
    wlo_in = nc.dram_tensor("wlo", [P, max(NL, 1) * 8], dt.int16, kind="ExternalInput")
    whi_in = nc.dram_tensor("whi", [P, max(NH, 1) * 8], dt.int16, kind="ExternalInput")
    dpjL_in = nc.dram_tensor("dpjL", [P, max(NL, 1)], dt.float32, kind="ExternalInput")
    dpjH_in = nc.dram_tensor("dpjH", [P, max(NH, 1)], dt.float32, kind="ExternalInput")
    djpL_in = nc.dram_tensor("djpL", [1, max(NL, 1) * 128], dt.bfloat16, kind="ExternalInput")
    djpH_in = nc.dram_tensor("djpH", [1, max(NH, 1) * 128], dt.bfloat16, kind="ExternalInput")
    out_t = nc.dram_tensor("out", [SHP, HEADS * C], dt.float32, kind="ExternalOutput")

    with tile.TileContext(nc) as tc, ExitStack() as ctx:
        const = ctx.enter_context(tc.tile_pool(name="const", bufs=1))

        # ---- constants
        iota_row_f = const.tile([P, P], dt.float32)
        nc.gpsimd.iota(iota_row_f[:], pattern=[[1, P]], base=0,
                       channel_multiplier=0, allow_small_or_imprecise_dtypes=True)
        iota_row = const.tile([P, P], dt.bfloat16)
        nc.vector.tensor_copy(out=iota_row[:], in_=iota_row_f[:])
        iota_col4 = const.tile([P, 1024], dt.float32)
        nc.gpsimd.iota(iota_col4[:], pattern=[[0, 1024]], base=0,
                       channel_multiplier=1, allow_small_or_imprecise_dtypes=True)
        iota_colb = const.tile([P, 1024], dt.bfloat16)
        nc.vector.tensor_copy(out=iota_colb[:], in_=iota_col4[:])
        ones1 = const.tile([1, P], dt.bfloat16)
        nc.gpsimd.memset(ones1[:], 1.0)
        ones_col = const.tile([P, 1], dt.bfloat16)
        nc.gpsimd.memset(ones_col[:], 1.0)
        wa4_sb = const.tile([P, 4], dt.bfloat16)
        nc.sync.dma_start(wa4_sb[:], wa4_in.ap()[:, :])
        wsb = const.tile([P, HEADS * C], dt.bfloat16)
        nc.sync.dma_start(wsb[:], wsb_in.ap()[:, :])
        bias_bc = const.tile([P, HEADS * C], dt.float32)
        nc.sync.dma_start(bias_bc[:], bass.AP(bias_in, 0, [[0, P], [1, HEADS * C]]))
        adst_sb = const.tile([P, NBLK, 2], dt.float32)
        adst_bf = const.tile([P, NBLK, 2], dt.bfloat16)

        # ---- phase B input tables (preload during phase A)
        wloT = const.tile([P, max(NL, 1) * 8], dt.int16)
        nc.sync.dma_start(wloT[:], wlo_in.ap()[:, :])
        whiT = const.tile([P, max(NH, 1) * 8], dt.int16)
        nc.sync.dma_start(whiT[:], whi_in.ap()[:, :])
        dpjL_sb = const.tile([P, max(NL, 1)], dt.float32)
        nc.sync.dma_start(dpjL_sb[:], dpjL_in.ap()[:, :])
        dpjH_sb = const.tile([P, max(NH, 1)], dt.float32)
        nc.sync.dma_start(dpjH_sb[:], dpjH_in.ap()[:, :])

        # ---- phase A: a4 = hT_tile^T @ wa4 for all N; embed a_src into hpack
        ctxA = ExitStack()
        sbA = ctxA.enter_context(tc.tile_pool(name="sbA", bufs=3))
        psA = ctxA.enter_context(tc.tile_pool(name="psA", bufs=3, space="PSUM"))
        st = const.tile([P, NTILE, 4], dt.float32)   # a4 staging, all tiles
        GA = 96                     # h tiles per hT DMA
        t = 0
        while t < NTILE:
            nt = min(GA, NTILE - t)
            ht = sbA.tile([P, GA * 128], dt.bfloat16, tag="ht")
            nc.sync.dma_start(ht[:, :nt * 128],
                              hT_in.ap()[:, t * 128:(t + nt) * 128])
            for g0 in range(0, nt, 4):
                n4 = min(4, nt - g0)
                a4p = psA.tile([P, 4, 4], dt.float32, tag="a4", space="PSUM")
                for g in range(n4):
                    nc.tensor.matmul(out=a4p[:, g, :],
                                     lhsT=ht[:, (g0 + g) * 128:(g0 + g + 1) * 128],
                                     rhs=wa4_sb[:], start=True, stop=True)
                nc.scalar.activation(out=st[:, t + g0:t + g0 + n4, :],
                                      in_=a4p[:, :n4, :], func=act.Copy)
            emb_ap = bass.AP(hpack, t * 128 * 128 + 64,
                             [[128, P], [128 * 128, nt], [1, 2]])
            nc.sync.dma_start(emb_ap, st[:, t:t + nt, 0:2])
            t += nt


        # phase A-bis: local shard a_dst from h_shT
        hts = sbA.tile([P, SHP], dt.bfloat16, tag="hts")
        nc.sync.dma_start(hts[:], hshT_in.ap()[:, :])
        for t4 in range(13):
            nt = min(4, NBLK - t4 * 4)
            if nt <= 0:
                break
            a4p = psA.tile([P, 4, 2], dt.float32, tag="a4b", space="PSUM")
            for g in range(nt):
                nc.tensor.matmul(out=a4p[:, g, :],
                                 lhsT=hts[:, (t4 * 4 + g) * 128:(t4 * 4 + g + 1) * 128],
                                 rhs=wa4_sb[:, 2:4], start=True, stop=True)
            nc.scalar.activation(out=adst_sb[:, t4 * 4:t4 * 4 + nt, :],
                                  in_=a4p[:, :nt, :], func=act.Copy)
        nc.vector.tensor_copy(out=adst_bf[:], in_=adst_sb[:])
        ctxA.close()

        # ---- phase B
        ghL = ctx.enter_context(tc.tile_pool(name="ghL", bufs=2))
        ghH = ctx.enter_context(tc.tile_pool(name="ghH", bufs=2))
        gdj = ctx.enter_context(tc.tile_pool(name="gdj", bufs=2))
        mk = ctx.enter_context(tc.tile_pool(name="mk", bufs=3))
        sm = ctx.enter_context(tc.tile_pool(name="sm", bufs=2))
        fin = ctx.enter_context(tc.tile_pool(name="fin", bufs=2))
        psGT = ctx.enter_context(tc.tile_pool(name="psGT", bufs=2, space="PSUM"))
        psAD = ctx.enter_context(tc.tile_pool(name="psAD", bufs=2, space="PSUM"))
        psSS = ctx.enter_context(tc.tile_pool(name="psSS", bufs=1, space="PSUM"))
        psU = ctx.enter_context(tc.tile_pool(name="psU", bufs=1, space="PSUM"))

        hp_ap = hpack.ap()
        for (b0, b1) in runs:
            nL = offL[b1] - offL[b0]
            nH = offH[b1] - offH[b0]
            hgl = ghL.tile([P, RL, 128], dt.float32, tag="hgl")
            if nL:
                nc.gpsimd.dma_gather(
                    out_ap=hgl[:, :nL, :], in_ap=hp_ap[0:LO, :],
                    idxs_ap=wloT[:, offL[b0] * 8:offL[b1] * 8],
                    num_idxs=nL * P, num_idxs_reg=nL * P,
                    elem_size=128, single_packet=False)
            hgh = ghH.tile([P, RH, 128], dt.float32, tag="hgh")
            if nH:
                nc.gpsimd.dma_gather(
                    out_ap=hgh[:, :nH, :], in_ap=hp_ap[LO:NPAD, :],
                    idxs_ap=whiT[:, offH[b0] * 8:offH[b1] * 8],
                    num_idxs=nH * P, num_idxs_reg=nH * P,
                    elem_size=128, single_packet=False)
            djl = gdj.tile([1, RL * 128], dt.bfloat16, tag="djl")
            if nL:
                nc.sync.dma_start(djl[:, :nL * 128],
                                  djpL_in.ap()[:, offL[b0] * 128:offL[b1] * 128])
            djh = gdj.tile([1, RH * 128], dt.bfloat16, tag="djh")
            if nH:
                nc.sync.dma_start(djh[:, :nH * 128],
                                  djpH_in.ap()[:, offH[b0] * 128:offH[b1] * 128])

            for b in range(b0, b1):
                kl, kh = KL[b], KH[b]
                K = kl + kh
                if K == 0:
                    continue
                # chunk descriptors: (hg tile, col in tile, dpj table, pos)
                chunks = []
                for j in range(kl):
                    chunks.append((hgl, offL[b] - offL[b0] + j, dpjL_sb,
                                   offL[b] + j, djl, (offL[b] - offL[b0] + j)))
                for j in range(kh):
                    chunks.append((hgh, offH[b] - offH[b0] + j, dpjH_sb,
                                   offH[b] + j, djh, (offH[b] - offH[b0] + j)))

                # pass 1: a_dst per slot via mask-transpose matmuls
                adp = psAD.tile([P, KMAX, 2], dt.float32, tag="adp", space="PSUM")
                for g0 in range(0, K, 8):
                    ng = min(8, K - g0)
                    dbc = mk.tile([P, 8 * 128], dt.bfloat16, tag="dbc")
                    # pbc needs contiguous djl cols: lo and hi parts separate
                    done = 0
                    while done < ng:
                        djt, dcol = chunks[g0 + done][4], chunks[g0 + done][5]
                        nrun = 1
                        while (done + nrun < ng
                               and chunks[g0 + done + nrun][4] is djt
                               and chunks[g0 + done + nrun][5] == dcol + nrun):
                            nrun += 1
                        nc.gpsimd.partition_broadcast(
                            dbc[:, done * 128:(done + nrun) * 128],
                            djt[0:1, dcol * 128:(dcol + nrun) * 128])
                        done += nrun
                    mt4 = mk.tile([P, 8 * 128], dt.bfloat16, tag="mt4")
                    nc.vector.tensor_tensor(
                        out=mt4[:, :ng * 128], in0=iota_colb[:, :ng * 128],
                        in1=dbc[:, :ng * 128], op=op.is_equal)
                    for gg in range(ng):
                        nc.tensor.matmul(
                            out=adp[:, g0 + gg, :],
                            lhsT=mt4[:, gg * 128:(gg + 1) * 128],
                            rhs=adst_bf[:, b, :], start=True, stop=True)

                # logits -> ex  [P, K, 2]
                tsum = sm.tile([P, KMAX, 2], dt.float32, tag="tsum")
                if kl:
                    nc.vector.tensor_tensor(
                        out=tsum[:, :kl, :],
                        in0=hgl[:, offL[b] - offL[b0]:offL[b] - offL[b0] + kl, 64:66],
                        in1=adp[:, :kl, :], op=op.add)
                if kh:
                    nc.vector.tensor_tensor(
                        out=tsum[:, kl:K, :],
                        in0=hgh[:, offH[b] - offH[b0]:offH[b] - offH[b0] + kh, 64:66],
                        in1=adp[:, kl:K, :], op=op.add)
                u02 = sm.tile([P, KMAX, 2], dt.float32, tag="u02")
                nc.vector.tensor_scalar(out=u02[:, :K, :], in0=tsum[:, :K, :],
                                        scalar1=NEG_SLOPE, scalar2=None, op0=op.mult)
                lr = sm.tile([P, KMAX, 2], dt.float32, tag="lr")
                nc.vector.tensor_tensor(out=lr[:, :K, :], in0=tsum[:, :K, :],
                                        in1=u02[:, :K, :], op=op.max)
                ex2 = sm.tile([P, KMAX, 2], dt.float32, tag="ex2")
                nc.scalar.activation(out=ex2[:, :K, :], in_=lr[:, :K, :], func=act.Exp)

                # pass 2: masked scatter matmuls
                gtt = psGT.tile([P, HEADS * P], dt.float32, tag="gtt", space="PSUM")
                ss0 = psSS.tile([P, 1], dt.float32, tag="ss0", space="PSUM")
                ss1 = psSS.tile([P, 1], dt.float32, tag="ss1", space="PSUM")
                for k, (hg, col, dpjt, dpos, _, _) in enumerate(chunks):
                    st_, sp_ = k == 0, k == K - 1
                    exm = mk.tile([P, 2 * P], dt.bfloat16, tag="exm")
                    nc.vector.tensor_scalar(
                        out=exm[:, 0:P], in0=iota_row[:],
                        scalar1=dpjt[:, dpos:dpos + 1],
                        scalar2=ex2[:, k, 0:1], op0=op.is_equal, op1=op.mult)
                    nc.vector.tensor_scalar(
                        out=exm[:, P:2 * P], in0=iota_row[:],
                        scalar1=dpjt[:, dpos:dpos + 1],
                        scalar2=ex2[:, k, 1:2], op0=op.is_equal, op1=op.mult)
                    nc.tensor.matmul(out=gtt[:], lhsT=hg[:, col, 0:64].bitcast(dt.bfloat16),
                                     rhs=exm[:], start=st_, stop=sp_)
                    nc.tensor.matmul(out=ss0[:], lhsT=exm[:, 0:P], rhs=ones_col[:],
                                     start=st_, stop=sp_)
                    nc.tensor.matmul(out=ss1[:], lhsT=exm[:, P:2 * P], rhs=ones_col[:],
                                     start=st_, stop=sp_)

                # ---- finalize block b
                rec = fin.tile([P, 2], dt.float32, tag="rec")
                nc.vector.reciprocal(out=rec[:, 0:1], in_=ss0[:])
                nc.vector.reciprocal(out=rec[:, 1:2], in_=ss1[:])
                ob = fin.tile([P, HEADS * C], dt.float32, tag="ob")
                for hd in range(HEADS):
                    gs = fin.tile([P, P], dt.bfloat16, tag="gs")
                    nc.scalar.activation(out=gs[:], in_=gtt[:, hd * P:(hd + 1) * P],
                                         func=act.Copy)
                    u = psU.tile([P, C], dt.float32, tag="u", space="PSUM")
                    nc.tensor.matmul(out=u[:], lhsT=gs[:],
                                     rhs=wsb[:, hd * C:(hd + 1) * C],
                                     start=True, stop=True)
                    o2 = fin.tile([P, C], dt.float32, tag="o2")
                    nc.scalar.activation(out=o2[:], in_=u[:], func=act.Copy,
                                         scale=rec[:, hd:hd + 1])
                    o3 = fin.tile([P, C], dt.float32, tag="o3")
                    nc.vector.tensor_tensor(out=o3[:], in0=o2[:],
                                            in1=bias_bc[:, hd * C:(hd + 1) * C],
                                            op=op.add)
                    rl = fin.tile([P, C], dt.float32, tag="rl")
                    nc.scalar.activation(out=rl[:], in_=o3[:], func=act.Relu,
                                         scale=-1.0)
                    e1 = fin.tile([P, C], dt.float32, tag="e1")
                    nc.scalar.activation(out=e1[:], in_=rl[:], func=act.Exp,
                                         scale=-1.0)
                    a3 = fin.tile([P, C], dt.float32, tag="a3")
                    nc.vector.tensor_scalar(out=a3[:], in0=o3[:], scalar1=0.0,
                                            scalar2=-1.0, op0=op.max, op1=op.add)
                    nc.vector.tensor_tensor(out=ob[:, hd * C:(hd + 1) * C],
                                            in0=a3[:], in1=e1[:], op=op.add)
                nc.sync.dma_start(out_t.ap()[b * P:(b + 1) * P, :], ob[:])

    nc.compile()
    return nc


def _get_program(params):
    if params not in _CACHE:
        _CACHE[params] = _build(params)
    return _CACHE[params]


# ------------------------------------------------------------------- kernel
def kernel(h_node, edge_index, W, att_src, att_dst, bias):
    from concourse.bass_utils import run_bass_kernel_spmd

    h_node = np.asarray(h_node, dtype=np.float32)
    W = np.asarray(W, dtype=np.float32)
    att_src = np.asarray(att_src, dtype=np.float32)
    att_dst = np.asarray(att_dst, dtype=np.float32)
    bias = np.asarray(bias, dtype=np.float32)

    params, wlo, whi, dpjL_t, dpjH_t, djpL, djpH = _prep(np.asarray(edge_index))
    hpack, hT, h_shT, wa4, wsb, bias2 = _pack_inputs(h_node, W, att_src, att_dst, bias)
    nc = _get_program(params)

    KLs, KHs, _ = params
    NL = sum(KLs)
    NH = sum(KHs)
    in_maps = []
    for c in range(NC_CORES):
        in_maps.append({
            "hpack": hpack, "hT": hT, "hshT": h_shT[c], "wa4": wa4,
            "wsb": wsb, "bias_in": bias2,
            "wlo": wlo[c], "whi": whi[c],
            "dpjL": dpjL_t[c], "dpjH": dpjH_t[c],
            "djpL": djpL[c].reshape(1, NL * 128),
            "djpH": djpH[c].reshape(1, NH * 128),
        })
    res = run_bass_kernel_spmd(nc, in_maps, core_ids=list(range(NC_CORES)))
    out = np.concatenate([res.results[c]["out"][:SH] for c in range(NC_CORES)], axis=0)
    return out


# revision 16
# speedup vs baseline: 2.9076x; 1.0051x over previous
"""GAT layer (PyG GATConv eval, 2 heads x 128, self-loops, ELU) on 8 trn2 cores.

v2 strategy (dst-sharded, per core):
  hpack[N,128] f32 rows (512B): cols 0:64 = h as packed bf16 pairs (host),
  cols 64:66 = a_src logits f32 (device phase A embeds).  ONE dma_gather by
  src id per edge slot fetches h (bf16) + a_src together.
  Phase A: a4 = hT_tile^T @ wa4 (host passes hT bf16, no PE transposes),
  embed a_src into hpack; local-shard a_dst kept in SBUF (h_shT input).
  Phase B: edges grouped by dst block (128 dsts), chunks of 128 slots,
  per-block chunk counts = max over cores (not global max).  Per chunk:
  exm = fused is_equal+mult masks (bf16), gtt += hg^T @ exm (bf16 PE),
  segsum via exm^T @ ones, a_dst per slot via mask-transpose matmuls.
  Finalize: U = GT^T W (bf16), normalize, +bias, exact ELU.
"""
import math
from contextlib import ExitStack

import numpy as np
import ml_dtypes

BF16 = ml_dtypes.bfloat16
HEADS = 2
C = 128
IN = 128
N = 50000
NC_CORES = 8
SH = N // NC_CORES            # 6250 dst nodes per core
NBLK = math.ceil(SH / 128)    # 49 dst blocks per core
SHP = NBLK * 128              # padded shard rows 6272
NTILE = math.ceil(N / 128)    # 391 tiles of full h
NPAD = NTILE * 128            # 50048 padded rows of hpack
LO = 32768                    # int16 gather index split
RL = 48                       # max lo chunks per gather run
RH = 28                       # max hi chunks per gather run
NEG_SLOPE = 0.2

_CACHE = {}


# ----------------------------------------------------------------- host prep
def _wrap16(idx, nchunk):
    """idx [nchunk*128] int16 -> wrapped gather table [128, nchunk*8]."""
    sl = idx.reshape(nchunk * 8, 16)            # [col, p16]
    w = np.broadcast_to(sl.T[None, :, :], (8, 16, nchunk * 8))
    return np.ascontiguousarray(w.reshape(128, nchunk * 8))


def _prep(edge_index):
    src = np.concatenate([edge_index[0], np.arange(N, dtype=np.int64)])
    dst = np.concatenate([edge_index[1], np.arange(N, dtype=np.int64)])
    core = dst // SH
    blk = (dst % SH) // 128
    dloc = (dst % SH) % 128
    half = (src >= LO).astype(np.int64)

    # per (core, block, half) counts -> per-block chunk counts (max over cores)
    cnt = np.zeros((NC_CORES, NBLK, 2), dtype=np.int64)
    np.add.at(cnt, (core, blk, half), 1)
    KL = np.maximum(np.ceil(cnt[:, :, 0] / 128).astype(np.int64).max(0), 0)
    KH = np.maximum(np.ceil(cnt[:, :, 1] / 128).astype(np.int64).max(0), 0)
    offL = np.concatenate([[0], np.cumsum(KL)])   # lo-stream chunk offsets
    offH = np.concatenate([[0], np.cumsum(KH)])
    NL, NH = int(offL[-1]), int(offH[-1])

    # slot assignment: stable sort by (core, blk, half); rank within group
    key = (core * NBLK + blk) * 2 + half
    order = np.argsort(key, kind="stable")
    key_s = key[order]
    sizes = np.bincount(key_s, minlength=NC_CORES * NBLK * 2)
    starts = np.concatenate([[0], np.cumsum(sizes)[:-1]])
    rank = np.arange(len(key_s)) - starts[key_s]
    src_s = src[order]
    dloc_s = dloc[order]
    core_s = key_s // (2 * NBLK)
    blk_s = (key_s // 2) % NBLK
    half_s = key_s % 2

    # global slot position within each core's lo/hi stream
    strm_off = np.where(half_s == 0, offL[blk_s] * 128, offH[blk_s] * 128)
    slot = strm_off + rank

    idxL = np.zeros((NC_CORES, NL * 128), dtype=np.int16)
    idxH = np.zeros((NC_CORES, NH * 128), dtype=np.int16)
    dpjL = np.full((NC_CORES, NL, 128), 999.0, dtype=np.float32)
    dpjH = np.full((NC_CORES, NH, 128), 999.0, dtype=np.float32)

    lo_m = half_s == 0
    idxL[core_s[lo_m], slot[lo_m]] = src_s[lo_m].astype(np.int16)
    idxH[core_s[~lo_m], slot[~lo_m]] = (src_s[~lo_m] - LO).astype(np.int16)
    dpjL[core_s[lo_m], slot[lo_m] // 128, slot[lo_m] % 128] = dloc_s[lo_m]
    dpjH[core_s[~lo_m], slot[~lo_m] // 128, slot[~lo_m] % 128] = dloc_s[~lo_m]

    wlo = np.stack([_wrap16(idxL[c], NL) for c in range(NC_CORES)])
    whi = np.stack([_wrap16(idxH[c], NH) for c in range(NC_CORES)])
    # dpj tables [128 partitions, nchunk] (scalar per partition per chunk)
    dpjL_t = np.ascontiguousarray(dpjL.transpose(0, 2, 1))
    dpjH_t = np.ascontiguousarray(dpjH.transpose(0, 2, 1))
    # djp rows [1, nchunk*128] bf16 for the PE broadcast matmul
    djpL = dpjL.reshape(NC_CORES, NL * 128).astype(BF16)
    djpH = dpjH.reshape(NC_CORES, NH * 128).astype(BF16)

    # gather runs: greedy whole blocks with sum KL<=RL and sum KH<=RH
    runs = []
    b = 0
    while b < NBLK:
        b1 = b + 1
        while b1 < NBLK and (KL[b:b1 + 1].sum() <= RL and KH[b:b1 + 1].sum() <= RH):
            b1 += 1
        runs.append((b, b1))
        b = b1
    params = (tuple(int(k) for k in KL), tuple(int(k) for k in KH),
              tuple(runs))
    return params, wlo, whi, dpjL_t, dpjH_t, djpL, djpH


def _pack_inputs(h_node, W, att_src, att_dst, bias):
    hb = h_node.astype(BF16)                       # [N,128] bf16
    hpack = np.zeros((NPAD, 128), dtype=np.float32)
    hpack[:N, 0:64] = hb.view(np.uint16).reshape(N, 64, 2).view(np.uint32).reshape(N, 64).view(np.float32)
    hT = np.zeros((128, NPAD), dtype=BF16)
    hT[:, :N] = hb.T
    h_shT = np.zeros((NC_CORES, 128, SHP), dtype=BF16)
    for c in range(NC_CORES):
        h_shT[c, :, :SH] = hb[c * SH:(c + 1) * SH].T
    W3 = W.reshape(IN, HEADS, C)
    wa4 = np.stack([
        np.einsum('cho,ho->c', W3, att_src * (np.arange(HEADS)[:, None] == 0)),
        np.einsum('cho,ho->c', W3, att_src * (np.arange(HEADS)[:, None] == 1)),
        np.einsum('cho,ho->c', W3, att_dst * (np.arange(HEADS)[:, None] == 0)),
        np.einsum('cho,ho->c', W3, att_dst * (np.arange(HEADS)[:, None] == 1)),
    ], axis=1).astype(BF16)                        # [128, 4]
    wsb = W.astype(BF16)                           # [128, 256]
    bias2 = bias.reshape(1, HEADS * C).astype(np.float32)
    return hpack, hT, h_shT, wa4, wsb, bias2


# ------------------------------------------------------------ device program
def _build(params):
    import concourse.bacc as bacc
    import concourse.bass as bass
    import concourse.mybir as mybir
    import concourse.tile as tile

    KL, KH, runs = params
    offL = [0]
    for k in KL:
        offL.append(offL[-1] + k)
    offH = [0]
    for k in KH:
        offH.append(offH[-1] + k)
    NL, NH = offL[-1], offH[-1]
    KMAX = max(KL[b] + KH[b] for b in range(NBLK))

    dt = mybir.dt
    op = mybir.AluOpType
    act = mybir.ActivationFunctionType
    P = 128

    nc = bacc.Bacc("TRN2", target_bir_lowering=False, debug=False,
                   num_devices=NC_CORES)
    hpack = nc.dram_tensor("hpack", [NPAD, 128], dt.float32, kind="ExternalInput")
    hT_in = nc.dram_tensor("hT", [128, NPAD], dt.bfloat16, kind="ExternalInput")
    hshT_in = nc.dram_tensor("hshT", [128, SHP], dt.bfloat16, kind="ExternalInput")
    wa4_in = nc.dram_tensor("wa4", [128, 4], dt.bfloat16, kind="ExternalInput")
    wsb_in = nc.dram_tensor("wsb", [128, HEADS * C], dt.bfloat16, kind="ExternalInput")
    bias_in = nc.dram_tensor("bias_in", [1, HEADS * C], dt.float32, kind="ExternalInput")
    wlo_in = nc.dram_tensor("wlo", [P, max(NL, 1) * 8], dt.int16, kind="ExternalInput")
    whi_in = nc.dram_tensor("whi", [P, max(NH, 1) * 8], dt.int16, kind="ExternalInput")
    dpjL_in = nc.dram_tensor("dpjL", [P, max(NL, 1)], dt.float32, kind="ExternalInput")
    dpjH_in = nc.dram_tensor("dpjH", [P, max(NH, 1)], dt.float32, kind="ExternalInput")
    djpL_in = nc.dram_tensor("djpL", [1, max(NL, 1) * 128], dt.bfloat16, kind="ExternalInput")
    djpH_in = nc.dram_tensor("djpH", [1, max(NH, 1) * 128], dt.bfloat16, kind="ExternalInput")
    out_t = nc.dram_tensor("out", [SHP, HEADS * C], dt.float32, kind="ExternalOutput")

    with tile.TileContext(nc) as tc, ExitStack() as ctx:
        const = ctx.enter_context(tc.tile_pool(name="const", bufs=1))

        # ---- constants
        iota_row_f = const.tile([P, P], dt.float32)
        nc.gpsimd.iota(iota_row_f[:], pattern=[[1, P]], base=0,
                       channel_multiplier=0, allow_small_or_imprecise_dtypes=True)
        iota_row = const.tile([P, P], dt.bfloat16)
        nc.vector.tensor_copy(out=iota_row[:], in_=iota_row_f[:])
        iota_col4 = const.tile([P, 1024], dt.float32)
        nc.gpsimd.iota(iota_col4[:], pattern=[[0, 1024]], base=0,
                       channel_multiplier=1, allow_small_or_imprecise_dtypes=True)
        iota_colb = const.tile([P, 1024], dt.bfloat16)
        nc.vector.tensor_copy(out=iota_colb[:], in_=iota_col4[:])
        ones1 = const.tile([1, P], dt.bfloat16)
        nc.gpsimd.memset(ones1[:], 1.0)
        ones_col = const.tile([P, 1], dt.bfloat16)
        nc.gpsimd.memset(ones_col[:], 1.0)
        wa4_sb = const.tile([P, 4], dt.bfloat16)
        nc.sync.dma_start(wa4_sb[:], wa4_in.ap()[:, :])
        wsb = const.tile([P, HEADS * C], dt.bfloat16)
        nc.sync.dma_start(wsb[:], wsb_in.ap()[:, :])
        bias_bc = const.tile([P, HEADS * C], dt.float32)
        nc.sync.dma_start(bias_bc[:], bass.AP(bias_in, 0, [[0, P], [1, HEADS * C]]))
        adst_sb = const.tile([P, NBLK, 2], dt.float32)
        adst_bf = const.tile([P, NBLK, 2], dt.bfloat16)

        # ---- phase B input tables (preload during phase A)
        wloT = const.tile([P, max(NL, 1) * 8], dt.int16)
        nc.sync.dma_start(wloT[:], wlo_in.ap()[:, :])
        whiT = const.tile([P, max(NH, 1) * 8], dt.int16)
        nc.sync.dma_start(whiT[:], whi_in.ap()[:, :])
        dpjL_sb = const.tile([P, max(NL, 1)], dt.float32)
        nc.sync.dma_start(dpjL_sb[:], dpjL_in.ap()[:, :])
        dpjH_sb = const.tile([P, max(NH, 1)], dt.float32)
        nc.sync.dma_start(dpjH_sb[:], dpjH_in.ap()[:, :])

        # ---- phase A: a4 = hT_tile^T @ wa4 for all N; embed a_src into hpack
        ctxA = ExitStack()
        sbA = ctxA.enter_context(tc.tile_pool(name="sbA", bufs=3))
        psA = ctxA.enter_context(tc.tile_pool(name="psA", bufs=3, space="PSUM"))
        st = const.tile([P, NTILE, 4], dt.float32)   # a4 staging, all tiles
        GA = 96                     # h tiles per hT DMA
        t = 0
        while t < NTILE:
            nt = min(GA, NTILE - t)
            ht = sbA.tile([P, GA * 128], dt.bfloat16, tag="ht")
            nc.sync.dma_start(ht[:, :nt * 128],
                              hT_in.ap()[:, t * 128:(t + nt) * 128])
            for g0 in range(0, nt, 4):
                n4 = min(4, nt - g0)
                a4p = psA.tile([P, 4, 4], dt.float32, tag="a4", space="PSUM")
                for g in range(n4):
                    nc.tensor.matmul(out=a4p[:, g, :],
                                     lhsT=ht[:, (g0 + g) * 128:(g0 + g + 1) * 128],
                                     rhs=wa4_sb[:], start=True, stop=True)
                nc.scalar.activation(out=st[:, t + g0:t + g0 + n4, :],
                                      in_=a4p[:, :n4, :], func=act.Copy)
            emb_ap = bass.AP(hpack, t * 128 * 128 + 64,
                             [[128, P], [128 * 128, nt], [1, 2]])
            nc.sync.dma_start(emb_ap, st[:, t:t + nt, 0:2])
            t += nt


        # phase A-bis: local shard a_dst from h_shT
        hts = sbA.tile([P, SHP], dt.bfloat16, tag="hts")
        nc.sync.dma_start(hts[:], hshT_in.ap()[:, :])
        for t4 in range(13):
            nt = min(4, NBLK - t4 * 4)
            if nt <= 0:
                break
            a4p = psA.tile([P, 4, 2], dt.float32, tag="a4b", space="PSUM")
            for g in range(nt):
                nc.tensor.matmul(out=a4p[:, g, :],
                                 lhsT=hts[:, (t4 * 4 + g) * 128:(t4 * 4 + g + 1) * 128],
                                 rhs=wa4_sb[:, 2:4], start=True, stop=True)
            nc.scalar.activation(out=adst_sb[:, t4 * 4:t4 * 4 + nt, :],
                                  in_=a4p[:, :nt, :], func=act.Copy)
        nc.vector.tensor_copy(out=adst_bf[:], in_=adst_sb[:])
        ctxA.close()

        # ---- phase B
        ghL = ctx.enter_context(tc.tile_pool(name="ghL", bufs=2))
        ghH = ctx.enter_context(tc.tile_pool(name="ghH", bufs=2))
        gdj = ctx.enter_context(tc.tile_pool(name="gdj", bufs=2))
        mk = ctx.enter_context(tc.tile_pool(name="mk", bufs=3))
        sm = ctx.enter_context(tc.tile_pool(name="sm", bufs=2))
        fin = ctx.enter_context(tc.tile_pool(name="fin", bufs=2))
        psGT = ctx.enter_context(tc.tile_pool(name="psGT", bufs=2, space="PSUM"))
        psAD = ctx.enter_context(tc.tile_pool(name="psAD", bufs=2, space="PSUM"))
        psSS = ctx.enter_context(tc.tile_pool(name="psSS", bufs=1, space="PSUM"))
        psU = ctx.enter_context(tc.tile_pool(name="psU", bufs=1, space="PSUM"))

        hp_ap = hpack.ap()
        for (b0, b1) in runs:
            nL = offL[b1] - offL[b0]
            nH = offH[b1] - offH[b0]
            hgl = ghL.tile([P, RL, 128], dt.float32, tag="hgl")
            if nL:
                nc.gpsimd.dma_gather(
                    out_ap=hgl[:, :nL, :], in_ap=hp_ap[0:LO, :],
                    idxs_ap=wloT[:, offL[b0] * 8:offL[b1] * 8],
                    num_idxs=nL * P, num_idxs_reg=nL * P,
                    elem_size=128, single_packet=False)
            hgh = ghH.tile([P, RH, 128], dt.float32, tag="hgh")
            if nH:
                nc.gpsimd.dma_gather(
                    out_ap=hgh[:, :nH, :], in_ap=hp_ap[LO:NPAD, :],
                    idxs_ap=whiT[:, offH[b0] * 8:offH[b1] * 8],
                    num_idxs=nH * P, num_idxs_reg=nH * P,
                    elem_size=128, single_packet=False)
            djl = gdj.tile([1, RL * 128], dt.bfloat16, tag="djl")
            if nL:
                nc.sync.dma_start(djl[:, :nL * 128],
                                  djpL_in.ap()[:, offL[b0] * 128:offL[b1] * 128])
            djh = gdj.tile([1, RH * 128], dt.bfloat16, tag="djh")
            if nH:
                nc.sync.dma_start(djh[:, :nH * 128],
                                  djpH_in.ap()[:, offH[b0] * 128:offH[b1] * 128])

            for b in range(b0, b1):
                kl, kh = KL[b], KH[b]
                K = kl + kh
                if K == 0:
                    continue
                # chunk descriptors: (hg tile, col in tile, dpj table, pos)
                chunks = []
                for j in range(kl):
                    chunks.append((hgl, offL[b] - offL[b0] + j, dpjL_sb,
                                   offL[b] + j, djl, (offL[b] - offL[b0] + j)))
                for j in range(kh):
                    chunks.append((hgh, offH[b] - offH[b0] + j, dpjH_sb,
                                   offH[b] + j, djh, (offH[b] - offH[b0] + j)))

                # pass 1: a_dst per slot via mask-transpose matmuls
                adp = psAD.tile([P, KMAX, 2], dt.float32, tag="adp", space="PSUM")
                for g0 in range(0, K, 8):
                    ng = min(8, K - g0)
                    dbc = mk.tile([P, 8 * 128], dt.bfloat16, tag="dbc")
                    # pbc needs contiguous djl cols: lo and hi parts separate
                    done = 0
                    while done < ng:
                        djt, dcol = chunks[g0 + done][4], chunks[g0 + done][5]
                        nrun = 1
                        while (done + nrun < ng
                               and chunks[g0 + done + nrun][4] is djt
                               and chunks[g0 + done + nrun][5] == dcol + nrun):
                            nrun += 1
                        nc.gpsimd.partition_broadcast(
                            dbc[:, done * 128:(done + nrun) * 128],
                            djt[0:1, dcol * 128:(dcol + nrun) * 128])
                        done += nrun
                    mt4 = mk.tile([P, 8 * 128], dt.bfloat16, tag="mt4")
                    nc.vector.tensor_tensor(
                        out=mt4[:, :ng * 128], in0=iota_colb[:, :ng * 128],
                        in1=dbc[:, :ng * 128], op=op.is_equal)
                    for gg in range(ng):
                        nc.tensor.matmul(
                            out=adp[:, g0 + gg, :],
                            lhsT=mt4[:, gg * 128:(gg + 1) * 128],
                            rhs=adst_bf[:, b, :], start=True, stop=True)

                # logits -> ex  [P, K, 2]
                tsum = sm.tile([P, KMAX, 2], dt.float32, tag="tsum")
                if kl:
                    nc.vector.tensor_tensor(
                        out=tsum[:, :kl, :],
                        in0=hgl[:, offL[b] - offL[b0]:offL[b] - offL[b0] + kl, 64:66],
                        in1=adp[:, :kl, :], op=op.add)
                if kh:
                    nc.vector.tensor_tensor(
                        out=tsum[:, kl:K, :],
                        in0=hgh[:, offH[b] - offH[b0]:offH[b] - offH[b0] + kh, 64:66],
                        in1=adp[:, kl:K, :], op=op.add)
                u02 = sm.tile([P, KMAX, 2], dt.float32, tag="u02")
                nc.vector.tensor_scalar(out=u02[:, :K, :], in0=tsum[:, :K, :],
                                        scalar1=NEG_SLOPE, scalar2=None, op0=op.mult)
                lr = sm.tile([P, KMAX, 2], dt.float32, tag="lr")
                nc.vector.tensor_tensor(out=lr[:, :K, :], in0=tsum[:, :K, :],
                                        in1=u02[:, :K, :], op=op.max)
                ex2 = sm.tile([P, KMAX, 2], dt.float32, tag="ex2")
                nc.scalar.activation(out=ex2[:, :K, :], in_=lr[:, :K, :], func=act.Exp)

                # pass 2: masked scatter matmuls
                gtt = psGT.tile([P, HEADS * P], dt.float32, tag="gtt", space="PSUM")
                ss0 = psSS.tile([P, 1], dt.float32, tag="ss0", space="PSUM")
                ss1 = psSS.tile([P, 1], dt.float32, tag="ss1", space="PSUM")
                for k, (hg, col, dpjt, dpos, _, _) in enumerate(chunks):
                    st_, sp_ = k == 0, k == K - 1
                    exm = mk.tile([P, 2 * P], dt.bfloat16, tag="exm")
                    nc.vector.tensor_scalar(
                        out=exm[:, 0:P], in0=iota_row[:],
                        scalar1=dpjt[:, dpos:dpos + 1],
                        scalar2=ex2[:, k, 0:1], op0=op.is_equal, op1=op.mult)
                    nc.vector.tensor_scalar(
                        out=exm[:, P:2 * P], in0=iota_row[:],
                        scalar1=dpjt[:, dpos:dpos + 1],
                        scalar2=ex2[:, k, 1:2], op0=op.is_equal, op1=op.mult)
                    nc.tensor.matmul(out=gtt[:], lhsT=hg[:, col, 0:64].bitcast(dt.bfloat16),
                                     rhs=exm[:], start=st_, stop=sp_)
                    nc.tensor.matmul(out=ss0[:], lhsT=exm[:, 0:P], rhs=ones_col[:],
                                     start=st_, stop=sp_)
                    nc.tensor.matmul(out=ss1[:], lhsT=exm[:, P:2 * P], rhs=ones_col[:],
                                     start=st_, stop=sp_)

                # ---- finalize block b
                rec = fin.tile([P, 2], dt.float32, tag="rec")
                nc.vector.reciprocal(out=rec[:, 0:1], in_=ss0[:])
                nc.vector.reciprocal(out=rec[:, 1:2], in_=ss1[:])
                ob = fin.tile([P, HEADS * C], dt.float32, tag="ob")
                for hd in range(HEADS):
                    gs = fin.tile([P, P], dt.bfloat16, tag="gs")
                    nc.scalar.activation(out=gs[:], in_=gtt[:, hd * P:(hd + 1) * P],
                                         func=act.Copy)
                    u = psU.tile([P, C], dt.float32, tag="u", space="PSUM")
                    nc.tensor.matmul(out=u[:], lhsT=gs[:],
                                     rhs=wsb[:, hd * C:(hd + 1) * C],
                                     start=True, stop=True)
                    o2 = fin.tile([P, C], dt.float32, tag="o2")
                    nc.scalar.activation(out=o2[:], in_=u[:], func=act.Copy,
                                         scale=rec[:, hd:hd + 1])
                    o3 = fin.tile([P, C], dt.float32, tag="o3")
                    nc.vector.tensor_tensor(out=o3[:], in0=o2[:],
                                            in1=bias_bc[:, hd * C:(hd + 1) * C],
                                            op=op.add)
                    rl = fin.tile([P, C], dt.float32, tag="rl")
                    nc.scalar.activation(out=rl[:], in_=o3[:], func=act.Relu,
                                         scale=-1.0)
                    e1 = fin.tile([P, C], dt.float32, tag="e1")
                    nc.scalar.activation(out=e1[:], in_=rl[:], func=act.Exp,
                                         scale=-1.0)
                    r2 = fin.tile([P, C], dt.float32, tag="r2")
                    nc.scalar.activation(out=r2[:], in_=o3[:], func=act.Relu)
                    nc.vector.scalar_tensor_tensor(
                        out=ob[:, hd * C:(hd + 1) * C], in0=e1[:], scalar=-1.0,
                        in1=r2[:], op0=op.add, op1=op.add)
                nc.sync.dma_start(out_t.ap()[b * P:(b + 1) * P, :], ob[:])

    nc.compile()
    return nc


def _get_program(params):
    if params not in _CACHE:
        _CACHE[params] = _build(params)
    return _CACHE[params]


# ------------------------------------------------------------------- kernel
def kernel(h_node, edge_index, W, att_src, att_dst, bias):
    from concourse.bass_utils import run_bass_kernel_spmd

    h_node = np.asarray(h_node, dtype=np.float32)
    W = np.asarray(W, dtype=np.float32)
    att_src = np.asarray(att_src, dtype=np.float32)
    att_dst = np.asarray(att_dst, dtype=np.float32)
    bias = np.asarray(bias, dtype=np.float32)

    params, wlo, whi, dpjL_t, dpjH_t, djpL, djpH = _prep(np.asarray(edge_index))
    hpack, hT, h_shT, wa4, wsb, bias2 = _pack_inputs(h_node, W, att_src, att_dst, bias)
    nc = _get_program(params)

    KLs, KHs, _ = params
    NL = sum(KLs)
    NH = sum(KHs)
    in_maps = []
    for c in range(NC_CORES):
        in_maps.append({
            "hpack": hpack, "hT": hT, "hshT": h_shT[c], "wa4": wa4,
            "wsb": wsb, "bias_in": bias2,
            "wlo": wlo[c], "whi": whi[c],
            "dpjL": dpjL_t[c], "dpjH": dpjH_t[c],
            "djpL": djpL[c].reshape(1, NL * 128),
            "djpH": djpH[c].reshape(1, NH * 128),
        })
    res = run_bass_kernel_spmd(nc, in_maps, core_ids=list(range(NC_CORES)))
    out = np.concatenate([res.results[c]["out"][:SH] for c in range(NC_CORES)], axis=0)
    return out
